# revision 13
# baseline (speedup 1.0000x reference)
"""AdderNet (ResNet20-style, L1-distance convs) on 8 TRN2 NeuronCores.

Self-contained: kernel(**inputs) takes the full unsharded inputs and returns
the full [32, 10] float32 output. Data-parallel over the batch (4 images per
core); BatchNorm batch stats made exact via a per-conv AllGather of
(sum, sumsq) + local reduce.

v2 design (vs baseline):
  - D = |x - w| in ONE DVE op: tensor_scalar(subtract, abs_max 0)
  - PE column-tiling: psum rows 32*j hold different images/chunks, matmuls
    issued to 4 (or 2) distinct 32-column array groups run concurrently
  - stride-2 convs read from stride-1 "parity planes" (precompacted)
  - AllGather (floor ~5us) instead of AllReduce (~10us); the partition
    re-gather happens for free in the return DMA's access pattern
  - replication DMAs spread across engine queues, per-image granularity
"""

import numpy as np

CORES = 8
BL = 4          # local batch per core
EPS = 1e-5
GB = CORES * BL  # global batch

# per-conv D-op engine split: name -> (n_act, n_gps) ops routed off DVE.
# D-ops are ranked misaligned-first; first n_act go to ACT, next n_gps to GPS.
ENG_SPLIT = {}
for _b in range(3):
    ENG_SPLIT[f"l1b{_b}c1"] = (8, 16)
    ENG_SPLIT[f"l1b{_b}c2"] = (8, 16)
ENG_SPLIT["l2tc1"] = (12, 24)
ENG_SPLIT["l2td"] = (0, 4)
for _n in ("l2tc2", "l2b0c1", "l2b0c2", "l2b1c1", "l2b1c2"):
    ENG_SPLIT[_n] = (12, 24)
ENG_SPLIT["l3tc1"] = (30, 48)
ENG_SPLIT["l3td"] = (0, 0)
for _n in ("l3tc2", "l3b0c1", "l3b0c2", "l3b1c1", "l3b1c2"):
    ENG_SPLIT[_n] = (48, 72)


# --------------------------------------------------------------------------
# network schedule
# --------------------------------------------------------------------------
# cst variants: (ci, g, co). Mblk = min(co, 32).
CST_VARIANTS = [(16, 8, 16), (16, 8, 32), (32, 4, 32), (32, 4, 64), (64, 2, 64)]


def cst_layout():
    off = {}
    ones_off = {}
    o = 0
    for (ci, g, co) in CST_VARIANTS:
        mblk = min(co, 32)
        off[(ci, g, co)] = o
        o += (co // g) * mblk
        ones_off[(ci, g, co)] = o
        o += co
    return off, ones_off, o


def conv_meta(ci, co, hin, stride, k):
    g = 128 // ci
    ncb = co // g
    hout = hin // stride
    bl = BL * hout * hout
    idx = next(i for i, v in enumerate(CST_VARIANTS) if v == (ci, g, co))
    return dict(ci=ci, co=co, g=g, ncb=ncb, k=k, stride=stride,
                hin=hin, hout=hout, bl=bl, cst=idx, mblk=min(co, 32))


def make_schedule():
    convs = []

    def add(name, wsrc, ci, co, hin, stride, k, **roles):
        m = conv_meta(ci, co, hin, stride, k)
        m.update(name=name, wsrc=wsrc, **roles)
        convs.append(m)

    rot = [("X0", "X1", "X2"), ("X2", "X0", "X1"), ("X1", "X2", "X0")]
    for b in range(3):
        i, mid, o = rot[b]
        add(f"l1b{b}c1", ("l1_w", 2 * b), 16, 16, 32, 1, 3, inb=i, outb=mid, evac="relu")
        add(f"l1b{b}c2", ("l1_w", 2 * b + 1), 16, 16, 32, 1, 3, inb=mid, outb=o,
            evac="res", idb=i, idkind="pad")
    add("l2tc1", ("l2_w0",), 16, 32, 32, 2, 3, inb="X0", outb="Y0", evac="relu",
        grp="g2")
    add("l2td", ("l2_down",), 16, 32, 32, 2, 1, inb="X0", outb="ID2", evac="down",
        grp="g2")
    add("l2tc2", ("l2_ws", 0), 32, 32, 16, 1, 3, inb="Y0", outb="Y1", evac="res",
        idb="ID2", idkind="dense")
    rot2 = [("Y1", "Y2", "Y0"), ("Y0", "Y2", "Y1")]
    for b in range(2):
        i, mid, o = rot2[b]
        add(f"l2b{b}c1", ("l2_ws", 1 + 2 * b), 32, 32, 16, 1, 3, inb=i, outb=mid, evac="relu")
        add(f"l2b{b}c2", ("l2_ws", 2 + 2 * b), 32, 32, 16, 1, 3, inb=mid, outb=o,
            evac="res", idb=i, idkind="pad")
    add("l3tc1", ("l3_w0",), 32, 64, 16, 2, 3, inb="Y1", outb="Z0", evac="relu",
        grp="g3")
    add("l3td", ("l3_down",), 32, 64, 16, 2, 1, inb="Y1", outb="ID3", evac="down",
        grp="g3")
    add("l3tc2", ("l3_ws", 0), 64, 64, 8, 1, 3, inb="Z0", outb="Z1", evac="res",
        idb="ID3", idkind="dense")
    rot3 = [("Z1", "Z2", "Z0"), ("Z0", "Z2", "Z1")]
    for b in range(2):
        i, mid, o = rot3[b]
        add(f"l3b{b}c1", ("l3_ws", 1 + 2 * b), 64, 64, 8, 1, 3, inb=i, outb=mid, evac="relu")
        add(f"l3b{b}c2", ("l3_ws", 2 + 2 * b), 64, 64, 8, 1, 3, inb=mid, outb=o,
            evac="res", idb=i, idkind="pad")
    return convs


def d_ops(meta):
    """Yield (cb, s) in emission order. For co=64, interleave the two halves
    so consecutive matmuls target alternating PE column groups."""
    ncb, k = meta["ncb"], meta["k"]
    if meta["co"] == 64:
        nh = ncb // 2
        for q in range(nh):
            for s in range(k * k):
                yield q, s
                yield nh + q, s
    else:
        for cb in range(ncb):
            for s in range(k * k):
                yield cb, s


def d_misaligned(meta, cb, s):
    """True if the DVE src view for this op starts 2-byte-odd (drops 4x)."""
    k, stride = meta["k"], meta["stride"]
    kh, kw = divmod(s, k)
    if stride == 1:
        return kw % 2 == 1
    # parity-plane read: offset kw//2 within an even-width plane
    return (kw // 2) % 2 == 1


def d_engine_map(meta):
    """op index (position in d_ops order) -> 'V'/'A'/'G'."""
    n_act, n_gps = ENG_SPLIT.get(meta["name"], (0, 0))
    ops = list(d_ops(meta))
    # subchunk multiplicity: engine decided per (cb, s); counts are in subchunk
    # units, so convert: each (cb,s) has nsub sub-ops
    nsub = {16: BL, 32: BL // 2, 64: 1}[meta["co"]]
    order = sorted(range(len(ops)),
                   key=lambda i: (0 if d_misaligned(meta, *ops[i]) else 1, i))
    eng = {}
    a_left, g_left = n_act, n_gps
    for i in order:
        if a_left >= nsub:
            eng[i] = "A"
            a_left -= nsub
        elif g_left >= nsub:
            eng[i] = "G"
            g_left -= nsub
        else:
            eng[i] = "V"
    return ops, eng, nsub


SCHED = make_schedule()
NWALL = sum(c["ncb"] * c["k"] * c["k"] for c in SCHED)
CST_OFF, CST_ONES, NCST = cst_layout()


# --------------------------------------------------------------------------
# host-side packing
# --------------------------------------------------------------------------
def get_w(inputs, wsrc):
    a = inputs[wsrc[0]]
    if len(wsrc) > 1:
        a = a[wsrc[1]]
    return a  # [co, ci, k, k]


def pack_host(inputs):
    wall = np.zeros((128, NWALL), np.float32)
    col = 0
    for m in SCHED:
        w = get_w(inputs, m["wsrc"])
        ci, g, k = m["ci"], m["g"], m["k"]
        for cb, s in d_ops(m):
            kh, kw = divmod(s, k)
            for gg in range(g):
                co = cb * g + gg
                wall[gg * ci:(gg + 1) * ci, col] = w[co, :, kh, kw]
            col += 1
    assert col == NWALL

    cst = np.zeros((128, NCST), np.float16)
    for (ci, g, co) in CST_VARIANTS:
        off = CST_OFF[(ci, g, co)]
        mblk = min(co, 32)
        ncb = co // g
        nper = mblk // g  # blocks per half-window
        for cb in range(ncb):
            q = cb % nper
            for gg in range(g):
                cst[gg * ci:(gg + 1) * ci, off + cb * mblk + q * g + gg] = -2.0
        oo = CST_ONES[(ci, g, co)]
        cst[:, oo:oo + co] = 1.0 / g

    stemw = inputs["conv1_w"].transpose(2, 3, 1, 0).reshape(27, 16).astype(np.float16)
    fcw = (inputs["fc_w"][:, :, 0, 0].T / 64.0).astype(np.float32)  # [64, 10]
    return wall, cst, stemw, fcw


# --------------------------------------------------------------------------
# graph builder
# --------------------------------------------------------------------------
_CACHE = {}


def build(debug=False):
    from concourse import bacc, mybir, tile

    F16, F32 = mybir.dt.float16, mybir.dt.float32
    A = mybir.AluOpType
    AF = mybir.ActivationFunctionType
    AX = mybir.AxisListType

    nc = bacc.Bacc("TRN2", target_bir_lowering=False, debug=False,
                   num_devices=CORES)
    xp_d = nc.dram_tensor("xp", [3, BL, 34, 34], F16, kind="ExternalInput")
    wall_d = nc.dram_tensor("wall", [128, NWALL], F32, kind="ExternalInput")
    cst_d = nc.dram_tensor("cst", [128, NCST], F16, kind="ExternalInput")
    stemw_d = nc.dram_tensor("stemw", [27, 16], F16, kind="ExternalInput")
    fcw_d = nc.dram_tensor("fcw", [64, 10], F32, kind="ExternalInput")
    out_d = nc.dram_tensor("out", [10, BL], F32, kind="ExternalOutput")
    dbg_d = {}
    if debug:
        for m in SCHED:
            shp = ([m["co"], BL, m["hout"] + 2, m["hout"] + 2]
                   if m["evac"] != "down" else [m["co"], BL, m["hout"], m["hout"]])
            dbg_d[m["name"]] = nc.dram_tensor(f'dbg_{m["name"]}', shp,
                                              F16, kind="ExternalOutput")
        dbg_d["stem"] = nc.dram_tensor("dbg_stem", [16, BL, 34, 34],
                                       F16, kind="ExternalOutput")

    with tile.TileContext(nc) as tc:
        import contextlib
        with contextlib.ExitStack() as ctx:
            pp = ctx.enter_context(tc.tile_pool(name="persist", bufs=1))
            dp = ctx.enter_context(tc.tile_pool(name="dtiles", bufs=6))
            sp = ctx.enter_context(tc.tile_pool(name="small", bufs=8))
            ppl = ctx.enter_context(tc.tile_pool(name="planes", bufs=4))
            psp = ctx.enter_context(tc.tile_pool(name="psum", bufs=8, space="PSUM"))
            drp = ctx.enter_context(tc.tile_pool(name="dram", bufs=4, space="DRAM"))

            wall = pp.tile([128, NWALL], F32, tag="wall")
            nwall = pp.tile([128, NWALL], F32, tag="nwall")
            cst = pp.tile([128, NCST], F16, tag="cst")
            stemw = pp.tile([27, 16], F16, tag="stemw")
            fcw = pp.tile([64, 10], F32, tag="fcw")
            epst = pp.tile([128, 1], F32, tag="epst")
            nc.sync.dma_start(wall[:], wall_d[:])
            nc.sync.dma_start(cst[:], cst_d[:])
            nc.sync.dma_start(stemw[:], stemw_d[:])
            nc.sync.dma_start(fcw[:], fcw_d[:])
            nc.vector.memset(epst[:], EPS)
            nc.vector.tensor_scalar(nwall[:], wall[:], -1.0, None, A.mult)

            # activation buffers (persistent, zeroed once => borders stay 0)
            bufs = {}
            for nm in ("X0", "X1", "X2"):
                bufs[nm] = pp.tile([128, BL, 34, 34], F16, name=nm, tag=nm)
            for nm in ("Y0", "Y1", "Y2"):
                bufs[nm] = pp.tile([128, BL, 18, 18], F16, name=nm, tag=nm)
            for nm in ("Z0", "Z1", "Z2"):
                bufs[nm] = pp.tile([128, BL, 10, 10], F16, name=nm, tag=nm)
            bufs["ID2"] = pp.tile([128, BL, 16, 16], F16, name="ID2", tag="ID2")
            bufs["ID3"] = pp.tile([128, BL, 8, 8], F16, name="ID3", tag="ID3")
            for nm in ("X0", "X1", "X2", "Y0", "Y1", "Y2", "Z0", "Z2", "Z1"):
                nc.vector.memset(bufs[nm][:], 0.0)

            # round-robin DMA queue picker for replication copies
            rq_engines = None
            rq_i = [0]

            def rqueue():
                e = rq_engines[rq_i[0] % len(rq_engines)]
                rq_i[0] += 1
                return e
            rq_engines = [nc.sync, nc.gpsimd, nc.scalar]

            # ---------------- BN helpers ----------------
            def bn_finish(gred, n, rr, rows):
                """gred: [rows, 2] (S1, S2) global sums tile. rr: [rows, 2]
                out (r, -m*r). All ops vectorized over partition rows."""
                mt = sp.tile([rows, 4], F32, tag="bnm", name="bnm")
                nc.vector.tensor_scalar(mt[:, 0:1], gred[:, 0:1], 1.0 / n, None, A.mult)
                nc.vector.tensor_tensor(mt[:, 1:2], mt[:, 0:1], mt[:, 0:1], A.mult)
                nc.vector.tensor_scalar(mt[:, 2:3], gred[:, 1:2], 1.0 / n,
                                        mt[:, 1:2], A.mult, A.subtract)
                nc.scalar.activation(mt[:, 3:4], mt[:, 2:3], AF.Sqrt,
                                     bias=epst[0:rows, 0:1])
                nc.vector.reciprocal(rr[:, 0:1], mt[:, 3:4])
                nc.vector.tensor_scalar(rr[:, 1:2], mt[:, 0:1], -1.0,
                                        rr[:, 0:1], A.mult, A.mult)

            def allgather(st_tiles):
                """st_tiles: list of (tile, nelem_f32). Returns DRAM agout tile
                + per-input offset list. agout layout: [8 ranks, sum(nelem)]."""
                tot = sum(n for _, n in st_tiles)
                sin = drp.tile([tot], F32, tag="agi", name="agi")
                offs = []
                o = 0
                for t, n in st_tiles:
                    nc.sync.dma_start(sin[o:o + n], t[:])
                    offs.append(o)
                    o += n
                sout = drp.tile([CORES, tot], F32, tag="ago", name="ago")
                nc.gpsimd.collective_compute(
                    "AllGather", A.bypass,
                    replica_groups=[list(range(CORES))],
                    ins=[sin.opt()], outs=[sout.opt()],
                )
                return sout, offs

            # ---------------- per-layout helpers ----------------
            # layouts keyed by co: how psum / stats / evac are organized.
            def psum_alloc(meta, name):
                co = meta["co"]
                if co == 16:
                    return [psp.tile([128, 512], F32, tag="ps", name=f"{name}_b{b}")
                            for b in range(2)]
                if co == 32:
                    return [psp.tile([128, 256], F32, tag="ps", name=f"{name}_b0")]
                return [psp.tile([64, 256], F32, tag="ps", name=f"{name}_b0")]

            def stats_emit(meta, psums, st):
                co = meta["co"]
                if co == 16:
                    for b in range(2):
                        jk = dp.tile([128, 512], F16, tag="junk", name="junk")
                        nc.vector.tensor_scalar(jk[:], psums[b][:], 0.0, None,
                                                A.add, A.add,
                                                accum_out=st[:, 2 * b:2 * b + 1])
                        nc.scalar.activation(jk[:], psums[b][:], AF.Square,
                                             accum_out=st[:, 2 * b + 1:2 * b + 2])
                elif co == 32:
                    jk = dp.tile([128, 256], F16, tag="junk", name="junk")
                    nc.vector.tensor_scalar(jk[:], psums[0][:], 0.0, None,
                                            A.add, A.add, accum_out=st[:, 0:1])
                    nc.scalar.activation(jk[:], psums[0][:], AF.Square,
                                         accum_out=st[:, 1:2])
                else:
                    jk = dp.tile([64, 256], F16, tag="junk64", name="junk")
                    nc.vector.tensor_scalar(jk[:], psums[0][:], 0.0, None,
                                            A.add, A.add, accum_out=st[:, 0:1])
                    nc.scalar.activation(jk[:], psums[0][:], AF.Square,
                                         accum_out=st[:, 1:2])

            def st_alloc(meta):
                co = meta["co"]
                if co == 16:
                    return sp.tile([128, 4], F32, tag="st4", name=f"st_{meta['name']}"), 384
                if co == 32:
                    return sp.tile([128, 3], F32, tag="st", name=f"st_{meta['name']}"), 384
                return sp.tile([64, 3], F32, tag="st64", name=f"st_{meta['name']}"), 192

            def st_finalize(meta, st):
                """For co16: combine the two banks' partial stats -> [128, 3]."""
                if meta["co"] != 16:
                    return st
                st2 = sp.tile([128, 3], F32, tag="st", name="st2")
                nc.vector.tensor_tensor(st2[:, 0:2], st[:, 0:2], st[:, 2:4], A.add)
                return st2

            def gather_reduce(meta, sout, off):
                """Gather the AG output into per-channel layout + reduce + bn.
                Blob layout per rank: flat st2 [rows, 2] (row-major).
                Returns rr tile ([128,2] for co<=32 replicated, [64,2] co=64)."""
                co = meta["co"]
                n = GB * meta["hout"] * meta["hout"]
                if co == 64:
                    gst = sp.tile([64, 8, 2], F32, tag="gst64", name="gst")
                    sv = sout[:, off:off + 192].rearrange(
                        "r (i k) -> i r k", i=64, k=3)[:, :, 0:2]
                    nc.sync.dma_start(gst[:], sv)
                    red = sp.tile([64, 2], F32, tag="red64", name="red")
                    nc.vector.tensor_reduce(
                        red[:], gst[:, :, :].rearrange("p r k -> p k r"),
                        AX.X, A.add)
                    rr = sp.tile([64, 2], F32, tag="rr64", name="rr")
                    bn_finish(red, n, rr, 64)
                    return rr
                nch = co  # channels live at rows 32j+0:co
                gst = sp.tile([nch, 4, 8, 2], F32, tag="gst", name="gst")
                for j in range(4):
                    sv = sout[:, off + 96 * j:off + 96 * j + 3 * nch].rearrange(
                        "r (i k) -> i r k", i=nch, k=3)[:, :, 0:2]
                    nc.sync.dma_start(gst[:, j, :, :], sv)
                red = sp.tile([nch, 2], F32, tag="red", name="red")
                nc.vector.tensor_reduce(
                    red[:], gst[:, :, :, :].rearrange("p j r k -> p k (j r)"),
                    AX.X, A.add)
                rr = sp.tile([128, 2], F32, tag="rr", name="rr")
                bn_finish(red, n, rr[0:nch, :], nch)
                for t, eng in ((1, nc.scalar), (2, nc.gpsimd), (3, nc.sync)):
                    eng.dma_start(rr[32 * t:32 * t + nch, :], rr[0:nch, :])
                return rr

            def evacuate(meta, psums, rr):
                """psum -> xout (+ per-image replication)."""
                co, hout = meta["co"], meta["hout"]
                xout = bufs[meta["outb"]]
                kind = meta["evac"]
                idt = bufs[meta["idb"]] if kind == "res" else None
                if co == 16:
                    for j in range(BL):
                        for b in range(2):
                            ps = psums[b][32 * j:32 * j + 16, :]
                            ov = xout[32 * j:32 * j + 16, j,
                                      1 + 16 * b:17 + 16 * b, 1:33]
                            rrs = rr[32 * j:32 * j + 16, :]
                            if kind == "res":
                                idv = idt[32 * j:32 * j + 16, j,
                                          1 + 16 * b:17 + 16 * b, 1:33]
                                t = dp.tile([128, 512], F16, tag="tres", name="tres")
                                ts = t[32 * j:32 * j + 16, :]
                                nc.vector.scalar_tensor_tensor(
                                    ts, ps, rrs[:, 0:1], idv, A.mult, A.add)
                                nc.scalar.activation(ov, ts, AF.Relu,
                                                     bias=rrs[:, 1:2])
                            else:
                                nc.scalar.activation(ov, ps, AF.Relu,
                                                     bias=rrs[:, 1:2],
                                                     scale=rrs[:, 0:1])
                        # replicate image j to the other 7 groups
                        src = xout[32 * j:32 * j + 16, j, :, :]
                        for gg in range(8):
                            if gg == 2 * j:
                                continue
                            rqueue().dma_start(
                                xout[16 * gg:16 * gg + 16, j, :, :], src)
                elif co == 32:
                    hp = hout + 2
                    for c in range(BL):
                        ps = psums[0][32 * c:32 * c + 32, :]
                        rrs = rr[32 * c:32 * c + 32, :]
                        if kind == "down":
                            ov = bufs["ID2"][32 * c:32 * c + 32, c, :, :]
                            nc.scalar.activation(ov, ps, AF.Identity,
                                                 bias=rrs[:, 1:2], scale=rrs[:, 0:1])
                            src = bufs["ID2"][32 * c:32 * c + 32, c, :, :]
                            dstbuf = bufs["ID2"]
                            sh = [hout, hout]
                        else:
                            ov = xout[32 * c:32 * c + 32, c, 1:1 + hout, 1:1 + hout]
                            if kind == "res":
                                if meta["idkind"] == "pad":
                                    idv = idt[32 * c:32 * c + 32, c,
                                              1:1 + hout, 1:1 + hout]
                                else:
                                    idv = idt[32 * c:32 * c + 32, c, :, :]
                                t = dp.tile([128, 256], F16, tag="tres32", name="tres")
                                ts = t[32 * c:32 * c + 32, :]
                                nc.vector.scalar_tensor_tensor(
                                    ts, ps, rrs[:, 0:1], idv, A.mult, A.add)
                                nc.scalar.activation(ov, ts, AF.Relu,
                                                     bias=rrs[:, 1:2])
                            else:
                                nc.scalar.activation(ov, ps, AF.Relu,
                                                     bias=rrs[:, 1:2],
                                                     scale=rrs[:, 0:1])
                            src = xout[32 * c:32 * c + 32, c, :, :]
                            dstbuf = xout
                            sh = [hp, hp]
                        for gg in range(4):
                            if gg == c:
                                continue
                            rqueue().dma_start(
                                dstbuf[32 * gg:32 * gg + 32, c, :, :], src)
                else:  # co == 64
                    ps = psums[0][:, :]
                    if kind == "down":
                        ov = bufs["ID3"][0:64, :, :, :]
                        nc.scalar.activation(ov, ps, AF.Identity,
                                             bias=rr[:, 1:2], scale=rr[:, 0:1])
                        nc.sync.dma_start(bufs["ID3"][64:128, :, :, :],
                                          bufs["ID3"][0:64, :, :, :])
                    else:
                        nim = hout * hout
                        if kind == "res":
                            t = dp.tile([64, 256], F16, tag="tres64", name="tres")
                            for b in range(BL):
                                if meta["idkind"] == "pad":
                                    idv = idt[0:64, b, 1:1 + hout, 1:1 + hout]
                                else:
                                    idv = idt[0:64, b, :, :]
                                nc.vector.scalar_tensor_tensor(
                                    t[:, nim * b:nim * b + nim],
                                    psums[0][:, nim * b:nim * b + nim],
                                    rr[:, 0:1], idv, A.mult, A.add)
                            for b in range(BL):
                                nc.scalar.activation(
                                    xout[0:64, b, 1:1 + hout, 1:1 + hout],
                                    t[:, nim * b:nim * b + nim],
                                    AF.Relu, bias=rr[:, 1:2])
                        else:
                            for b in range(BL):
                                nc.scalar.activation(
                                    xout[0:64, b, 1:1 + hout, 1:1 + hout],
                                    psums[0][:, nim * b:nim * b + nim],
                                    AF.Relu, bias=rr[:, 1:2], scale=rr[:, 0:1])
                        nc.sync.dma_start(xout[64:128, :, :, :],
                                          xout[0:64, :, :, :])
                if debug and meta["name"] in dbg_d:
                    if kind == "down":
                        db = bufs["ID2"] if co == 32 else bufs["ID3"]
                        nc.sync.dma_start(dbg_d[meta["name"]][:], db[0:co])
                    else:
                        nc.sync.dma_start(dbg_d[meta["name"]][:], xout[0:co])

            # ---------------- parity planes for stride-2 convs ----------------
            def make_planes(meta):
                """Precompact stride-2 input into 4 stride-1 parity planes."""
                xin = bufs[meta["inb"]]
                hin = meta["hin"]          # 32 or 16
                hh = hin // 2 + 1          # 17 or 9
                wpl = hh + 1               # even width
                planes = {}
                engs = [nc.vector, nc.gpsimd, nc.vector, nc.gpsimd]
                i = 0
                for pr in (0, 1):
                    for pc in (0, 1):
                        pl = ppl.tile([128, BL, hh, wpl], F16,
                                      tag=f"pl{hin}", name=f"pl{pr}{pc}")
                        src = xin[:, :, pr:pr + 2 * hh - 1:2, pc:pc + 2 * hh - 1:2]
                        engs[i % 4].tensor_scalar(pl[:, :, :, 0:hh], src, 0.0,
                                                  None, A.add)
                        i += 1
                        planes[(pr, pc)] = pl
                return planes

            # ---------------- adder conv core ----------------
            wall_col = [0]

            def adder_conv(meta, planes=None):
                ci, co, g, ncb, k = meta["ci"], meta["co"], meta["g"], meta["ncb"], meta["k"]
                hout, stride = meta["hout"], meta["stride"]
                mblk = meta["mblk"]
                xin = bufs[meta["inb"]]
                coff = CST_OFF[CST_VARIANTS[meta["cst"]]]
                ones_off = CST_ONES[CST_VARIANTS[meta["cst"]]]
                psums = psum_alloc(meta, meta["name"])
                ops, engmap, nsub = d_engine_map(meta)
                ncol = {16: 512, 32: 256, 64: 256}[co]
                nh = ncb // 2 if co == 64 else None

                def xview(kh, kw, sub):
                    if stride == 2:
                        if k == 1:
                            pl, r0, c0 = planes[(1, 1)], 0, 0
                        else:
                            pl = planes[(kh % 2, kw % 2)]
                            r0, c0 = kh // 2, kw // 2
                        if co == 32 and sub is not None:  # per image-pair
                            p, = sub
                            return pl[:, 2 * p:2 * p + 2, r0:r0 + hout, c0:c0 + hout]
                        return pl[:, :, r0:r0 + hout, c0:c0 + hout]
                    if co == 16:
                        j, = sub
                        return xin[:, j, kh:kh + hout, kw:kw + hout]
                    if co == 32:
                        if sub is None:
                            return xin[:, :, kh:kh + hout, kw:kw + hout]
                        p, = sub
                        return xin[:, 2 * p:2 * p + 2, kh:kh + hout, kw:kw + hout]
                    return xin[:, :, kh:kh + hout, kw:kw + hout]

                def emit_d(eng, dv, xv, col):
                    # max-form: D = max(x, w) (DVE/GPS) or relu(x - w) (ACT);
                    # blockdiag(-2) + a sum-x ones matmul recovers -sum|x-w|
                    # up to a per-channel constant absorbed by BN.
                    if eng == "A":
                        nc.scalar.activation(dv, xv, AF.Relu,
                                             bias=nwall[:, col:col + 1])
                    elif eng == "G":
                        nc.gpsimd.tensor_scalar(dv, xv, wall[:, col:col + 1],
                                                None, A.max)
                    else:
                        nc.vector.tensor_scalar(dv, xv, wall[:, col:col + 1],
                                                None, A.max)

                dshape = {16: [128, hout, hout], 32: [128, 2, hout, hout],
                          64: [128, BL, hout, hout]}[co]
                dtag = f"d{co}_{hout}"

                for oi, (cb, s) in enumerate(ops):
                    kh, kw = divmod(s, k)
                    col = wall_col[0]
                    wall_col[0] += 1
                    eng = engmap[oi]
                    lhsT = cst[:, coff + cb * mblk:coff + (cb + 1) * mblk]
                    if co == 64:
                        h = cb // nh
                        first = (cb % nh == 0) and s == 0
                        last = (cb % nh == nh - 1) and s == k * k - 1
                        d = dp.tile(dshape, F16, tag=dtag, name="d")
                        emit_d(eng, d[:], xview(kh, kw, ()), col)
                        nc.tensor.matmul(
                            psums[0][32 * h:32 * h + 32, :], lhsT, d[:],
                            start=first, stop=last, tile_position=(0, 32 * h))
                        if cb == nh:  # both halves started: sum-x correction
                            nc.tensor.matmul(
                                psums[0][0:64, :],
                                cst[:, ones_off:ones_off + 64],
                                xview(kh, kw, ()),
                                start=False, stop=False, tile_position=(0, 0))
                    elif co == 32:
                        first = cb == 0 and s == 0
                        last = cb == ncb - 1 and s == k * k - 1
                        for p in range(2):
                            d = dp.tile(dshape, F16, tag=dtag, name="d")
                            emit_d(eng, d[:], xview(kh, kw, (p,)), col)
                            for ii in range(2):
                                c = 2 * p + ii
                                nc.tensor.matmul(
                                    psums[0][32 * c:32 * c + 32, :], lhsT,
                                    d[:, ii, :, :],
                                    start=first, stop=last,
                                    tile_position=(0, 32 * c))
                        if cb == 0:
                            xv = xview(kh, kw, None)
                            for c in range(BL):
                                nc.tensor.matmul(
                                    psums[0][32 * c:32 * c + 32, :],
                                    cst[:, ones_off:ones_off + 32],
                                    xv[:, c, :, :],
                                    start=False, stop=False,
                                    tile_position=(0, 32 * c))
                    else:  # co == 16
                        first = cb == 0 and s == 0
                        last = cb == ncb - 1 and s == k * k - 1
                        for j in range(BL):
                            d = dp.tile(dshape, F16, tag=dtag, name="d")
                            emit_d(eng, d[:], xview(kh, kw, (j,)), col)
                            for b in range(2):
                                nc.tensor.matmul(
                                    psums[b][32 * j:32 * j + 16, :], lhsT,
                                    d[:, 16 * b:16 * b + 16, :],
                                    start=first, stop=last,
                                    tile_position=(0, 32 * j))
                            if cb == 0:
                                xv = xview(kh, kw, (j,))
                                for b in range(2):
                                    nc.tensor.matmul(
                                        psums[b][32 * j:32 * j + 16, :],
                                        cst[:, ones_off:ones_off + 16],
                                        xv[:, 16 * b:16 * b + 16, :],
                                        start=False, stop=False,
                                        tile_position=(0, 32 * j))
                return psums

            def conv_tail(meta, psums):
                st, nst = st_alloc(meta)
                stats_emit(meta, psums, st)
                st = st_finalize(meta, st)
                sout, offs = allgather([(st, nst)])
                rr = gather_reduce(meta, sout, offs[0])
                evacuate(meta, psums, rr)

            # ---------------- stem ----------------
            with nc.named_scope("stem"):
                pt = pp.tile([27, BL, 32, 32], F16, tag="pt")
                for s in range(9):
                    kh, kw = divmod(s, 3)
                    nc.sync.dma_start(pt[3 * s:3 * s + 3],
                                      xp_d[:, :, kh:kh + 32, kw:kw + 32])
                m_stem = conv_meta(16, 16, 32, 1, 3)
                m_stem.update(outb="X0", evac="relu", name="stem")
                ps_stem = psum_alloc(m_stem, "stem")
                for j in range(BL):
                    for b in range(2):
                        nc.tensor.matmul(
                            ps_stem[b][32 * j:32 * j + 16, :], stemw[:],
                            pt[:, j, 16 * b:16 * b + 16, :],
                            start=True, stop=True, tile_position=(0, 32 * j))
                st, nst = st_alloc(m_stem)
                stats_emit(m_stem, ps_stem, st)
                st = st_finalize(m_stem, st)
                sout, offs = allgather([(st, nst)])
                rr = gather_reduce(m_stem, sout, offs[0])
                evacuate(m_stem, ps_stem, rr)
                if debug:
                    nc.sync.dma_start(dbg_d["stem"][:], bufs["X0"][0:16])

            # ---------------- adder conv layers ----------------
            i = 0
            while i < len(SCHED):
                meta = SCHED[i]
                if meta.get("grp"):  # merged transition pair (tc1 + td)
                    meta2 = SCHED[i + 1]
                    with nc.named_scope(meta["name"]):
                        planes = make_planes(meta)
                        ps1 = adder_conv(meta, planes)
                    with nc.named_scope(meta2["name"]):
                        ps2 = adder_conv(meta2, planes)
                        st1, n1 = st_alloc(meta)
                        st2, n2 = st_alloc(meta2)
                        stats_emit(meta, ps1, st1)
                        stats_emit(meta2, ps2, st2)
                        st1 = st_finalize(meta, st1)
                        st2 = st_finalize(meta2, st2)
                        sout, offs = allgather([(st1, n1), (st2, n2)])
                        rr1 = gather_reduce(meta, sout, offs[0])
                        rr2 = gather_reduce(meta2, sout, offs[1])
                        evacuate(meta, ps1, rr1)
                        evacuate(meta2, ps2, rr2)
                    i += 2
                else:
                    with nc.named_scope(meta["name"]):
                        ps = adder_conv(meta)
                        conv_tail(meta, ps)
                    i += 1

            # ---------------- avgpool + fc + final bn ----------------
            with nc.named_scope("fc"):
                zf = bufs[SCHED[-1]["outb"]]
                pooled = sp.tile([64, BL], F32, tag="pool", name="pooled")
                junkp = dp.tile([64, 64], F16, tag="junkp", name="junkp")
                for b in range(BL):
                    nc.scalar.activation(junkp[:], zf[0:64, b, 1:9, 1:9],
                                         AF.Identity,
                                         accum_out=pooled[:, b:b + 1])
                ps_fc = psp.tile([10, BL], F32, tag="ps", name="ps_fc")
                nc.tensor.matmul(ps_fc[:, :], fcw[:], pooled[:], start=True, stop=True)
                st = sp.tile([10, 2], F32, tag="stfc", name="st_fc")
                junk = dp.tile([10, BL], F16, tag="junkfc", name="junk_fc")
                nc.scalar.activation(junk[:], ps_fc[:], AF.Identity,
                                     accum_out=st[:, 0:1])
                nc.scalar.activation(junk[:], ps_fc[:], AF.Square,
                                     accum_out=st[:, 1:2])
                sout, offs = allgather([(st, 20)])
                gst = sp.tile([10, 8, 2], F32, tag="gstfc", name="gst_fc")
                sv = sout[:, 0:20].rearrange("r (i k) -> i r k", i=10, k=2)
                nc.sync.dma_start(gst[:], sv)
                red = sp.tile([10, 2], F32, tag="redfc", name="red_fc")
                nc.vector.tensor_reduce(
                    red[:], gst[:, :, :].rearrange("p r k -> p k r"),
                    mybir.AxisListType.X, A.add)
                rr = sp.tile([10, 2], F32, tag="rrfc", name="rr_fc")
                bn_finish(red, GB, rr, 10)
                osb = sp.tile([10, BL], F32, tag="osb", name="osb")
                nc.scalar.activation(osb[:], ps_fc[:], AF.Identity,
                                     bias=rr[:, 1:2], scale=rr[:, 0:1])
                nc.sync.dma_start(out_d[:], osb[:])

    nc.compile()
    return nc


def get_nc(debug=False):
    key = f"nc{debug}"
    if key not in _CACHE:
        _CACHE[key] = build(debug)
    return _CACHE[key]


# --------------------------------------------------------------------------
# entry point
# --------------------------------------------------------------------------
def kernel(**inputs):
    from concourse.bass_utils import run_bass_kernel_spmd

    x = inputs["x"]  # [32, 3, 32, 32] f32
    wall, cst, stemw, fcw = pack_host(inputs)
    xpad = np.zeros((CORES, 3, BL, 34, 34), np.float16)
    xs = x.reshape(CORES, BL, 3, 32, 32).transpose(0, 2, 1, 3, 4)
    xpad[:, :, :, 1:33, 1:33] = xs.astype(np.float16)

    nc = get_nc()
    in_maps = [{"xp": xpad[i], "wall": wall, "cst": cst,
                "stemw": stemw, "fcw": fcw} for i in range(CORES)]
    res = run_bass_kernel_spmd(nc, in_maps, list(range(CORES)))
    out = np.concatenate([r["out"].T for r in res.results], axis=0)
    return out.astype(np.float32)


# revision 26
# speedup vs baseline: 3.7759x; 3.7759x over previous
"""AdderNet (ResNet20-style, L1-distance convs) on 8 TRN2 NeuronCores.

Self-contained: kernel(**inputs) takes the full unsharded inputs and returns
the full [32, 10] float32 output. Data-parallel over the batch (4 images per
core); BatchNorm batch stats made exact via a per-conv AllGather of
(sum, sumsq) + local reduce.

v2 design (vs baseline):
  - D = |x - w| in ONE DVE op: tensor_scalar(subtract, abs_max 0)
  - PE column-tiling: psum rows 32*j hold different images/chunks, matmuls
    issued to 4 (or 2) distinct 32-column array groups run concurrently
  - stride-2 convs read from stride-1 "parity planes" (precompacted)
  - AllGather (floor ~5us) instead of AllReduce (~10us); the partition
    re-gather happens for free in the return DMA's access pattern
  - replication DMAs spread across engine queues, per-image granularity
"""

import numpy as np

CORES = 8
BL = 4          # local batch per core
EPS = 1e-5
GB = CORES * BL  # global batch

# per-conv D-op engine split: name -> n_act sub-ops routed to ACT (rest DVE).
# GpSimd is never used for tensor ops (measured ~40x slower + SBUF contention).
ENG_SPLIT = {}
for _b in range(3):
    ENG_SPLIT[f"l1b{_b}c1"] = 12
    ENG_SPLIT[f"l1b{_b}c2"] = 12
ENG_SPLIT["l2tc1"] = 28
ENG_SPLIT["l2td"] = 0
for _n in ("l2tc2", "l2b0c1", "l2b0c2", "l2b1c1", "l2b1c2"):
    ENG_SPLIT[_n] = 28
ENG_SPLIT["l3tc1"] = 36
ENG_SPLIT["l3td"] = 0
for _n in ("l3tc2", "l3b0c1", "l3b0c2", "l3b1c1", "l3b1c2"):
    ENG_SPLIT[_n] = 56


# --------------------------------------------------------------------------
# network schedule
# --------------------------------------------------------------------------
# cst variants: (ci, g, co). Mblk = min(co, 32).
CST_VARIANTS = [(16, 8, 16), (16, 8, 32), (32, 4, 32), (32, 4, 64), (64, 2, 64)]


def cst_layout():
    off = {}
    ones_off = {}
    o = 0
    for (ci, g, co) in CST_VARIANTS:
        mblk = min(co, 32)
        off[(ci, g, co)] = o
        o += (co // g) * mblk
        ones_off[(ci, g, co)] = o
        o += co
    return off, ones_off, o


def conv_meta(ci, co, hin, stride, k):
    g = 128 // ci
    ncb = co // g
    hout = hin // stride
    bl = BL * hout * hout
    idx = next(i for i, v in enumerate(CST_VARIANTS) if v == (ci, g, co))
    return dict(ci=ci, co=co, g=g, ncb=ncb, k=k, stride=stride,
                hin=hin, hout=hout, bl=bl, cst=idx, mblk=min(co, 32))


def make_schedule():
    convs = []

    def add(name, wsrc, ci, co, hin, stride, k, **roles):
        m = conv_meta(ci, co, hin, stride, k)
        m.update(name=name, wsrc=wsrc, **roles)
        convs.append(m)

    rot = [("X0", "X1", "X2"), ("X2", "X0", "X1"), ("X1", "X2", "X0")]
    for b in range(3):
        i, mid, o = rot[b]
        add(f"l1b{b}c1", ("l1_w", 2 * b), 16, 16, 32, 1, 3, inb=i, outb=mid, evac="relu")
        add(f"l1b{b}c2", ("l1_w", 2 * b + 1), 16, 16, 32, 1, 3, inb=mid, outb=o,
            evac="res", idb=i, idkind="pad")
    add("l2tc1", ("l2_w0",), 16, 32, 32, 2, 3, inb="X0", outb="Y0", evac="relu",
        grp="g2")
    add("l2td", ("l2_down",), 16, 32, 32, 2, 1, inb="X0", outb="ID2", evac="down",
        grp="g2")
    add("l2tc2", ("l2_ws", 0), 32, 32, 16, 1, 3, inb="Y0", outb="Y1", evac="res",
        idb="ID2", idkind="dense")
    rot2 = [("Y1", "Y2", "Y0"), ("Y0", "Y2", "Y1")]
    for b in range(2):
        i, mid, o = rot2[b]
        add(f"l2b{b}c1", ("l2_ws", 1 + 2 * b), 32, 32, 16, 1, 3, inb=i, outb=mid, evac="relu")
        add(f"l2b{b}c2", ("l2_ws", 2 + 2 * b), 32, 32, 16, 1, 3, inb=mid, outb=o,
            evac="res", idb=i, idkind="pad")
    add("l3tc1", ("l3_w0",), 32, 64, 16, 2, 3, inb="Y1", outb="Z0", evac="relu",
        grp="g3")
    add("l3td", ("l3_down",), 32, 64, 16, 2, 1, inb="Y1", outb="ID3", evac="down",
        grp="g3")
    add("l3tc2", ("l3_ws", 0), 64, 64, 8, 1, 3, inb="Z0", outb="Z1", evac="res",
        idb="ID3", idkind="dense")
    rot3 = [("Z1", "Z2", "Z0"), ("Z0", "Z2", "Z1")]
    for b in range(2):
        i, mid, o = rot3[b]
        add(f"l3b{b}c1", ("l3_ws", 1 + 2 * b), 64, 64, 8, 1, 3, inb=i, outb=mid, evac="relu")
        add(f"l3b{b}c2", ("l3_ws", 2 + 2 * b), 64, 64, 8, 1, 3, inb=mid, outb=o,
            evac="res", idb=i, idkind="pad")
    return convs


S_ORDER3 = [0, 2, 3, 5, 6, 8, 1, 4, 7]  # kw==1 last


def d_ops(meta):
    """Yield (cb, s) in emission order. kw==1 shifts come last within each cb
    (they read the shifted shadow buffer, written after replication). For
    co=64, interleave the two halves so consecutive matmuls target
    alternating PE column groups."""
    ncb, k = meta["ncb"], meta["k"]
    s_order = S_ORDER3 if k == 3 else [0]
    if meta["co"] == 64:
        nh = ncb // 2
        for q in range(nh):
            for s in s_order:
                yield q, s
                yield nh + q, s
    else:
        for cb in range(ncb):
            for s in s_order:
                yield cb, s


def d_engine_map(meta):
    """op index (position in d_ops order) -> 'V'/'A'."""
    n_act = ENG_SPLIT.get(meta["name"], 0)
    ops = list(d_ops(meta))
    nsub = {16: BL // 2, 32: BL // 2, 64: 1}[meta["co"]]
    n_act_ops = n_act // nsub
    eng = {}
    if n_act_ops > 0:
        stride = max(1, len(ops) // n_act_ops)
        left = n_act_ops
        for i in range(len(ops)):
            if i % stride == 0 and left > 0:
                eng[i] = "A"
                left -= 1
            else:
                eng[i] = "V"
    else:
        eng = {i: "V" for i in range(len(ops))}
    return ops, eng, nsub


SCHED = make_schedule()
NWALL = sum(c["ncb"] * c["k"] * c["k"] for c in SCHED)
CST_OFF, CST_ONES, NCST = cst_layout()


# --------------------------------------------------------------------------
# host-side packing
# --------------------------------------------------------------------------
def get_w(inputs, wsrc):
    a = inputs[wsrc[0]]
    if len(wsrc) > 1:
        a = a[wsrc[1]]
    return a  # [co, ci, k, k]


def pack_host(inputs):
    wall = np.zeros((128, NWALL), np.float32)
    col = 0
    for m in SCHED:
        w = get_w(inputs, m["wsrc"])
        ci, g, k = m["ci"], m["g"], m["k"]
        for cb, s in d_ops(m):
            kh, kw = divmod(s, k)
            for gg in range(g):
                co = cb * g + gg
                wall[gg * ci:(gg + 1) * ci, col] = w[co, :, kh, kw]
            col += 1
    assert col == NWALL

    cst = np.zeros((128, NCST), np.float16)
    for (ci, g, co) in CST_VARIANTS:
        off = CST_OFF[(ci, g, co)]
        mblk = min(co, 32)
        ncb = co // g
        nper = mblk // g  # blocks per half-window
        for cb in range(ncb):
            q = cb % nper
            for gg in range(g):
                cst[gg * ci:(gg + 1) * ci, off + cb * mblk + q * g + gg] = -2.0
        oo = CST_ONES[(ci, g, co)]
        cst[:, oo:oo + co] = 1.0 / g

    stemw = inputs["conv1_w"].transpose(2, 3, 1, 0).reshape(27, 16).astype(np.float16)
    fcw = (inputs["fc_w"][:, :, 0, 0].T / 64.0).astype(np.float32)  # [64, 10]
    return wall, cst, stemw, fcw


# --------------------------------------------------------------------------
# graph builder
# --------------------------------------------------------------------------
_CACHE = {}


def build(debug=False):
    from concourse import bacc, mybir, tile

    F16, F32 = mybir.dt.float16, mybir.dt.float32
    A = mybir.AluOpType
    AF = mybir.ActivationFunctionType
    AX = mybir.AxisListType

    nc = bacc.Bacc("TRN2", target_bir_lowering=False, debug=False,
                   num_devices=CORES)
    xp_d = nc.dram_tensor("xp", [3, BL, 34, 34], F16, kind="ExternalInput")
    wall_d = nc.dram_tensor("wall", [128, NWALL], F32, kind="ExternalInput")
    cst_d = nc.dram_tensor("cst", [128, NCST], F16, kind="ExternalInput")
    stemw_d = nc.dram_tensor("stemw", [27, 16], F16, kind="ExternalInput")
    fcw_d = nc.dram_tensor("fcw", [64, 10], F32, kind="ExternalInput")
    out_d = nc.dram_tensor("out", [10, BL], F32, kind="ExternalOutput")
    dbg_d = {}
    if debug:
        for m in SCHED:
            shp = ([m["co"], BL, m["hout"] + 2, m["hout"] + 2]
                   if m["evac"] != "down" else [m["co"], BL, m["hout"], m["hout"]])
            dbg_d[m["name"]] = nc.dram_tensor(f'dbg_{m["name"]}', shp,
                                              F16, kind="ExternalOutput")
        dbg_d["stem"] = nc.dram_tensor("dbg_stem", [16, BL, 34, 34],
                                       F16, kind="ExternalOutput")

    with tile.TileContext(nc) as tc:
        import contextlib
        with contextlib.ExitStack() as ctx:
            pp = ctx.enter_context(tc.tile_pool(name="persist", bufs=1))
            dp = ctx.enter_context(tc.tile_pool(name="dtiles", bufs=6))
            sp = ctx.enter_context(tc.tile_pool(name="small", bufs=8))
            ppl = ctx.enter_context(tc.tile_pool(name="planes", bufs=6))
            psp = ctx.enter_context(tc.tile_pool(name="psum", bufs=8, space="PSUM"))
            drp = ctx.enter_context(tc.tile_pool(name="dram", bufs=4, space="DRAM"))

            wall = pp.tile([128, NWALL], F32, tag="wall")
            nwall = pp.tile([128, NWALL], F32, tag="nwall")
            cst = pp.tile([128, NCST], F16, tag="cst")
            stemw = pp.tile([27, 16], F16, tag="stemw")
            fcw = pp.tile([64, 10], F32, tag="fcw")
            epst = pp.tile([128, 1], F32, tag="epst")
            nc.sync.dma_start(wall[:], wall_d[:])
            nc.sync.dma_start(cst[:], cst_d[:])
            nc.sync.dma_start(stemw[:], stemw_d[:])
            nc.sync.dma_start(fcw[:], fcw_d[:])
            nc.vector.memset(epst[:], EPS)
            nc.vector.tensor_scalar(nwall[:], wall[:], -1.0, None, A.mult)

            # activation buffers (persistent, zeroed once => borders stay 0)
            bufs = {}
            for nm in ("X0", "X1", "X2"):
                bufs[nm] = pp.tile([128, BL, 34, 34], F16, name=nm, tag=nm)
            for nm in ("Y0", "Y1", "Y2"):
                bufs[nm] = pp.tile([128, BL, 18, 18], F16, name=nm, tag=nm)
            for nm in ("Z0", "Z1", "Z2"):
                bufs[nm] = pp.tile([128, BL, 10, 10], F16, name=nm, tag=nm)
            bufs["ID2"] = pp.tile([128, BL, 16, 16], F16, name="ID2", tag="ID2")
            bufs["ID3"] = pp.tile([128, BL, 8, 8], F16, name="ID3", tag="ID3")
            # shifted shadow copies (one column left) so kw==1 D-reads stay
            # 4-byte aligned for the DVE 4x mode
            for nm in ("X0", "X1", "X2"):
                bufs[nm + "s"] = pp.tile([128, BL, 34, 33], F16, name=nm + "s",
                                         tag=nm + "s")
            for nm in ("Y0", "Y1", "Y2"):
                bufs[nm + "s"] = pp.tile([128, BL, 18, 17], F16, name=nm + "s",
                                         tag=nm + "s")
            for nm in ("Z0", "Z1", "Z2"):
                bufs[nm + "s"] = pp.tile([128, BL, 10, 9], F16, name=nm + "s",
                                         tag=nm + "s")
            for nm in ("X0", "X1", "X2", "Y0", "Y1", "Y2", "Z0", "Z2", "Z1"):
                nc.vector.memset(bufs[nm][:], 0.0)
                nc.vector.memset(bufs[nm + "s"][:], 0.0)

            # round-robin DMA queue picker for replication copies
            rq_engines = None
            rq_i = [0]

            def rqueue():
                e = rq_engines[rq_i[0] % len(rq_engines)]
                rq_i[0] += 1
                return e
            rq_engines = [nc.sync, nc.gpsimd]

            # ---------------- BN helpers ----------------
            def bn_finish(gred, n, rr, rows):
                """gred: [rows, 2] (S1, S2) global sums tile. rr: [rows, 2]
                out (r, -m*r). ACT-heavy to minimize engine switches."""
                mt = sp.tile([rows, 4], F32, tag="bnm", name="bnm")
                nc.scalar.activation(mt[:, 0:1], gred[:, 0:1], AF.Identity,
                                     scale=1.0 / n)                  # m
                nc.scalar.activation(mt[:, 1:2], mt[:, 0:1], AF.Square)  # m^2
                nc.scalar.activation(mt[:, 2:3], gred[:, 1:2], AF.Square)
                # v + eps = S2/n - m^2 + eps  (Square(sqrt..) trick avoided:
                # use Identity with scale and bias AP)
                nc.vector.tensor_scalar(mt[:, 2:3], gred[:, 1:2], 1.0 / n,
                                        mt[:, 1:2], A.mult, A.subtract)
                nc.scalar.activation(mt[:, 3:4], mt[:, 2:3], AF.Sqrt,
                                     bias=epst[0:rows, 0:1])
                nc.vector.reciprocal(rr[:, 0:1], mt[:, 3:4])
                nc.vector.tensor_scalar(rr[:, 1:2], mt[:, 0:1], -1.0,
                                        rr[:, 0:1], A.mult, A.mult)

            def allgather(st_tiles):
                """st_tiles: list of (tile, nelem_f32). Returns DRAM agout tile
                + per-input offset list. agout layout: [8 ranks, sum(nelem)]."""
                tot = sum(n for _, n in st_tiles)
                sin = drp.tile([tot], F32, tag="agi", name="agi")
                offs = []
                o = 0
                for t, n in st_tiles:
                    nc.sync.dma_start(sin[o:o + n], t[:])
                    offs.append(o)
                    o += n
                sout = drp.tile([CORES, tot], F32, tag="ago", name="ago")
                nc.gpsimd.collective_compute(
                    "AllGather", A.bypass,
                    replica_groups=[list(range(CORES))],
                    ins=[sin.opt()], outs=[sout.opt()],
                )
                return sout, offs

            # ---------------- per-layout helpers ----------------
            # layouts keyed by co: how psum / stats / evac are organized.
            def psum_alloc(meta, name):
                co = meta["co"]
                if co == 16:
                    return [psp.tile([128, 512], F32, tag="ps", name=f"{name}_b{b}")
                            for b in range(2)]
                if co == 32:
                    return [psp.tile([128, 256], F32, tag="ps", name=f"{name}_b0")]
                return [psp.tile([64, 256], F32, tag="ps", name=f"{name}_b0")]

            def emit_bank_stats16(st, psum, b):
                jk = dp.tile([128, 512], F16, tag="junk", name="junk")
                nc.vector.tensor_scalar(jk[:], psum[:], 0.0, None,
                                        A.add, A.add,
                                        accum_out=st[:, 2 * b:2 * b + 1])
                nc.scalar.activation(jk[:], psum[:], AF.Square,
                                     accum_out=st[:, 2 * b + 1:2 * b + 2])

            def stats_emit(meta, psums, st):
                co = meta["co"]
                if co == 16:
                    for b in range(2):
                        emit_bank_stats16(st, psums[b], b)
                elif co == 32:
                    jk = dp.tile([128, 256], F16, tag="junk", name="junk")
                    nc.vector.tensor_scalar(jk[:], psums[0][:], 0.0, None,
                                            A.add, A.add, accum_out=st[:, 0:1])
                    nc.scalar.activation(jk[:], psums[0][:], AF.Square,
                                         accum_out=st[:, 1:2])
                else:
                    jk = dp.tile([64, 256], F16, tag="junk64", name="junk")
                    nc.vector.tensor_scalar(jk[:], psums[0][:], 0.0, None,
                                            A.add, A.add, accum_out=st[:, 0:1])
                    nc.scalar.activation(jk[:], psums[0][:], AF.Square,
                                         accum_out=st[:, 1:2])

            def st_alloc(meta):
                co = meta["co"]
                if co == 16:
                    return sp.tile([128, 4], F32, tag="st4", name=f"st_{meta['name']}"), 384
                if co == 32:
                    return sp.tile([128, 3], F32, tag="st", name=f"st_{meta['name']}"), 384
                return sp.tile([64, 3], F32, tag="st64", name=f"st_{meta['name']}"), 192

            def st_finalize(meta, st):
                """For co16: combine the two banks' partial stats -> [128, 3]."""
                if meta["co"] != 16:
                    return st
                st2 = sp.tile([128, 3], F32, tag="st", name="st2")
                nc.vector.tensor_tensor(st2[:, 0:2], st[:, 0:2], st[:, 2:4], A.add)
                return st2

            def gather_reduce(meta, sout, off):
                """Gather the AG output into per-channel layout + reduce + bn.
                Blob layout per rank: flat st2 [rows, 2] (row-major).
                Returns rr tile ([128,2] for co<=32 replicated, [64,2] co=64)."""
                co = meta["co"]
                n = GB * meta["hout"] * meta["hout"]
                if co == 64:
                    gst = sp.tile([64, 8, 2], F32, tag="gst64", name="gst")
                    sv = sout[:, off:off + 192].rearrange(
                        "r (i k) -> i r k", i=64, k=3)[:, :, 0:2]
                    nc.sync.dma_start(gst[:], sv)
                    red = sp.tile([64, 2], F32, tag="red64", name="red")
                    nc.vector.tensor_reduce(
                        red[:], gst[:, :, :].rearrange("p r k -> p k r"),
                        AX.X, A.add)
                    rr = sp.tile([64, 2], F32, tag="rr64", name="rr")
                    bn_finish(red, n, rr, 64)
                    return rr
                nch = co  # channels live at rows 32j+0:co
                gst = sp.tile([nch, 4, 8, 2], F32, tag="gst", name="gst")
                for j in range(4):
                    sv = sout[:, off + 96 * j:off + 96 * j + 3 * nch].rearrange(
                        "r (i k) -> i r k", i=nch, k=3)[:, :, 0:2]
                    (nc.sync if j % 2 == 0 else nc.gpsimd).dma_start(
                        gst[:, j, :, :], sv)
                red = sp.tile([nch, 2], F32, tag="red", name="red")
                nc.vector.tensor_reduce(
                    red[:], gst[:, :, :, :].rearrange("p j r k -> p k (j r)"),
                    AX.X, A.add)
                rr = sp.tile([128, 2], F32, tag="rr", name="rr")
                bn_finish(red, n, rr[0:nch, :], nch)
                for t, eng in ((1, nc.scalar), (2, nc.gpsimd), (3, nc.sync)):
                    eng.dma_start(rr[32 * t:32 * t + nch, :], rr[0:nch, :])
                return rr

            def evacuate(meta, psums, rr):
                """psum -> xout (+ per-image replication)."""
                co, hout = meta["co"], meta["hout"]
                xout = bufs[meta["outb"]]
                kind = meta["evac"]
                idt = bufs[meta["idb"]] if kind == "res" else None
                if co == 16:
                    for j in range(BL):
                        for b in range(2):
                            rg = 32 * (2 * (j % 2) + b)
                            ps = psums[j // 2][rg:rg + 16, :]
                            ov = xout[rg:rg + 16, j,
                                      1 + 16 * b:17 + 16 * b, 1:33]
                            rrs = rr[rg:rg + 16, :]
                            if kind == "res":
                                idv = idt[rg:rg + 16, j,
                                          1 + 16 * b:17 + 16 * b, 1:33]
                                t = dp.tile([128, 512], F16, tag="tres", name="tres")
                                ts = t[rg:rg + 16, :]
                                nc.vector.scalar_tensor_tensor(
                                    ts, ps, rrs[:, 0:1], idv, A.mult, A.add)
                                nc.scalar.activation(ov, ts, AF.Relu,
                                                     bias=rrs[:, 1:2])
                            else:
                                nc.scalar.activation(ov, ps, AF.Relu,
                                                     bias=rrs[:, 1:2],
                                                     scale=rrs[:, 0:1])
                        # assemble + replicate image j to all 8 groups:
                        # halves evacuated to row-groups rb (top) and rb+32
                        # (bottom) -> cross-copy, double, then 64->64
                        rb = 64 * (j % 2)
                        rqueue().dma_start(xout[rb:rb + 16, j, 17:33, :],
                                           xout[rb + 32:rb + 48, j, 17:33, :])
                        rqueue().dma_start(xout[rb + 32:rb + 48, j, 0:17, :],
                                           xout[rb:rb + 16, j, 0:17, :])
                        rqueue().dma_start(xout[rb + 16:rb + 32, j, :, :],
                                           xout[rb:rb + 16, j, :, :])
                        rqueue().dma_start(xout[rb + 48:rb + 64, j, :, :],
                                           xout[rb + 32:rb + 48, j, :, :])
                        ro = (rb + 64) % 128
                        rqueue().dma_start(xout[ro:ro + 64, j, :, :],
                                           xout[rb:rb + 64, j, :, :])
                        xsh = bufs.get(meta["outb"] + "s")
                        if xsh is not None:
                            rqueue().dma_start(xsh[:, j, :, 0:33],
                                               xout[:, j, :, 1:34])
                elif co == 32:
                    hp = hout + 2
                    for c in range(BL):
                        ps = psums[0][32 * c:32 * c + 32, :]
                        rrs = rr[32 * c:32 * c + 32, :]
                        if kind == "down":
                            ov = bufs["ID2"][32 * c:32 * c + 32, c, :, :]
                            nc.scalar.activation(ov, ps, AF.Identity,
                                                 bias=rrs[:, 1:2], scale=rrs[:, 0:1])
                            src = bufs["ID2"][32 * c:32 * c + 32, c, :, :]
                            dstbuf = bufs["ID2"]
                            sh = [hout, hout]
                        else:
                            ov = xout[32 * c:32 * c + 32, c, 1:1 + hout, 1:1 + hout]
                            if kind == "res":
                                if meta["idkind"] == "pad":
                                    idv = idt[32 * c:32 * c + 32, c,
                                              1:1 + hout, 1:1 + hout]
                                else:
                                    idv = idt[32 * c:32 * c + 32, c, :, :]
                                t = dp.tile([128, 256], F16, tag="tres32", name="tres")
                                ts = t[32 * c:32 * c + 32, :]
                                nc.vector.scalar_tensor_tensor(
                                    ts, ps, rrs[:, 0:1], idv, A.mult, A.add)
                                nc.scalar.activation(ov, ts, AF.Relu,
                                                     bias=rrs[:, 1:2])
                            else:
                                nc.scalar.activation(ov, ps, AF.Relu,
                                                     bias=rrs[:, 1:2],
                                                     scale=rrs[:, 0:1])
                            src = xout[32 * c:32 * c + 32, c, :, :]
                            dstbuf = xout
                            sh = [hp, hp]
                        rb = 32 * c
                        rp = rb ^ 32
                        rqueue().dma_start(dstbuf[rp:rp + 32, c, :, :], src)
                        rh = rb // 64 * 64
                        ro = rh ^ 64
                        rqueue().dma_start(dstbuf[ro:ro + 64, c, :, :],
                                           dstbuf[rh:rh + 64, c, :, :])
                        if kind != "down":
                            xsh = bufs.get(meta["outb"] + "s")
                            if xsh is not None:
                                rqueue().dma_start(xsh[:, c, :, 0:17],
                                                   xout[:, c, :, 1:18])
                else:  # co == 64
                    ps = psums[0][:, :]
                    if kind == "down":
                        ov = bufs["ID3"][0:64, :, :, :]
                        nc.scalar.activation(ov, ps, AF.Identity,
                                             bias=rr[:, 1:2], scale=rr[:, 0:1])
                        nc.sync.dma_start(bufs["ID3"][64:128, :, :, :],
                                          bufs["ID3"][0:64, :, :, :])
                    else:
                        nim = hout * hout
                        if kind == "res":
                            t = dp.tile([64, 256], F16, tag="tres64", name="tres")
                            for b in range(BL):
                                if meta["idkind"] == "pad":
                                    idv = idt[0:64, b, 1:1 + hout, 1:1 + hout]
                                else:
                                    idv = idt[0:64, b, :, :]
                                nc.vector.scalar_tensor_tensor(
                                    t[:, nim * b:nim * b + nim],
                                    psums[0][:, nim * b:nim * b + nim],
                                    rr[:, 0:1], idv, A.mult, A.add)
                            for b in range(BL):
                                nc.scalar.activation(
                                    xout[0:64, b, 1:1 + hout, 1:1 + hout],
                                    t[:, nim * b:nim * b + nim],
                                    AF.Relu, bias=rr[:, 1:2])
                        else:
                            for b in range(BL):
                                nc.scalar.activation(
                                    xout[0:64, b, 1:1 + hout, 1:1 + hout],
                                    psums[0][:, nim * b:nim * b + nim],
                                    AF.Relu, bias=rr[:, 1:2], scale=rr[:, 0:1])
                        nc.sync.dma_start(xout[64:128, :, :, :],
                                          xout[0:64, :, :, :])
                        xsh = bufs.get(meta["outb"] + "s")
                        if xsh is not None:
                            rqueue().dma_start(xsh[:, :, :, 0:9],
                                               xout[:, :, :, 1:10])
                if debug and meta["name"] in dbg_d:
                    if kind == "down":
                        db = bufs["ID2"] if co == 32 else bufs["ID3"]
                        nc.sync.dma_start(dbg_d[meta["name"]][:], db[0:co])
                    else:
                        nc.sync.dma_start(dbg_d[meta["name"]][:], xout[0:co])

            # ---------------- parity planes for stride-2 convs ----------------
            def make_planes(meta):
                """Precompact stride-2 input into 4 stride-1 parity planes."""
                xin = bufs[meta["inb"]]
                hin = meta["hin"]          # 32 or 16
                hh = hin // 2 + 1          # 17 or 9
                wpl = hh + 1               # even width
                planes = {}
                engs = [nc.vector, nc.gpsimd, nc.vector, nc.gpsimd]
                i = 0
                for pr in (0, 1):
                    for pc in (0, 1):
                        pl = ppl.tile([128, BL, hh, wpl], F16,
                                      tag=f"pl{hin}", name=f"pl{pr}{pc}")
                        src = xin[:, :, pr:pr + 2 * hh - 1:2, pc:pc + 2 * hh - 1:2]
                        engs[i % 4].tensor_scalar(pl[:, :, :, 0:hh], src, 0.0,
                                                  None, A.add)
                        i += 1
                        planes[(pr, pc)] = pl
                return planes

            # ---------------- adder conv core ----------------
            wall_col = [0]

            def adder_conv(meta, planes=None, stats_st=None):
                ci, co, g, ncb, k = meta["ci"], meta["co"], meta["g"], meta["ncb"], meta["k"]
                hout, stride = meta["hout"], meta["stride"]
                mblk = meta["mblk"]
                xin = bufs[meta["inb"]]
                coff = CST_OFF[CST_VARIANTS[meta["cst"]]]
                ones_off = CST_ONES[CST_VARIANTS[meta["cst"]]]
                psums = psum_alloc(meta, meta["name"])
                ops, engmap, nsub = d_engine_map(meta)
                ncol = {16: 512, 32: 256, 64: 256}[co]
                nh = ncb // 2 if co == 64 else None

                def xview(kh, kw, sub):
                    if stride == 2:
                        if k == 1:
                            pl, r0, c0 = planes[(1, 1)], 0, 0
                        else:
                            pl = planes[(kh % 2, kw % 2)]
                            r0, c0 = kh // 2, kw // 2
                        if co == 32 and sub is not None:  # per image-pair
                            p, = sub
                            return pl[:, 2 * p:2 * p + 2, r0:r0 + hout, c0:c0 + hout]
                        return pl[:, :, r0:r0 + hout, c0:c0 + hout]
                    if co == 16:
                        j, = sub
                        return xin[:, j, kh:kh + hout, kw:kw + hout]
                    if co == 32:
                        if sub is None:
                            return xin[:, :, kh:kh + hout, kw:kw + hout]
                        p, = sub
                        return xin[:, 2 * p:2 * p + 2, kh:kh + hout, kw:kw + hout]
                    return xin[:, :, kh:kh + hout, kw:kw + hout]

                def emit_d(eng, dv, xv, col):
                    # max-form: D = max(x, w) (DVE/GPS) or relu(x - w) (ACT);
                    # blockdiag(-2) + a sum-x ones matmul recovers -sum|x-w|
                    # up to a per-channel constant absorbed by BN.
                    if eng == "A":
                        nc.scalar.activation(dv, xv, AF.Relu,
                                             bias=nwall[:, col:col + 1])
                    elif eng == "G":
                        nc.gpsimd.tensor_scalar(dv, xv, wall[:, col:col + 1],
                                                None, A.max)
                    else:
                        nc.vector.tensor_scalar(dv, xv, wall[:, col:col + 1],
                                                None, A.max)

                dshape = {16: [128, hout, hout], 32: [128, 2, hout, hout],
                          64: [128, BL, hout, hout]}[co]
                dtag = f"d{co}_{hout}"

                if co == 16:
                    # pair-split: all ops for images {0,1} (bank 0), then
                    # images {2,3} (bank 1); per-bank stats emitted inline so
                    # the AllGather can start while pair 1 computes.
                    colbase = wall_col[0]
                    wall_col[0] += len(ops)
                    for p in range(2):
                        for oi, (cb, s) in enumerate(ops):
                            kh, kw = divmod(s, k)
                            col = colbase + oi
                            eng = engmap[oi]
                            lhsT = cst[:, coff + cb * mblk:coff + (cb + 1) * mblk]
                            first = oi == 0
                            last = oi == len(ops) - 1
                            d = dp.tile(dshape, F16, tag=dtag, name="d")
                            emit_d(eng, d[:], xview(kh, kw, (p,)), col)
                            for ii in range(2):
                                for b in range(2):
                                    rg = 32 * (2 * ii + b)
                                    nc.tensor.matmul(
                                        psums[p][rg:rg + 16, :], lhsT,
                                        d[:, ii, 16 * b:16 * b + 16, :],
                                        start=first, stop=last,
                                        tile_position=(0, rg))
                            if cb == 0:
                                xv = xview(kh, kw, (p,))
                                for ii in range(2):
                                    for b in range(2):
                                        rg = 32 * (2 * ii + b)
                                        nc.tensor.matmul(
                                            psums[p][rg:rg + 16, :],
                                            cst[:, ones_off:ones_off + 16],
                                            xv[:, ii, 16 * b:16 * b + 16, :],
                                            start=False, stop=False,
                                            tile_position=(0, rg))
                        if stats_st is not None:
                            emit_bank_stats16(stats_st, psums[p], p)
                    return psums

                for oi, (cb, s) in enumerate(ops):
                    kh, kw = divmod(s, k)
                    col = wall_col[0]
                    wall_col[0] += 1
                    eng = engmap[oi]
                    lhsT = cst[:, coff + cb * mblk:coff + (cb + 1) * mblk]
                    if co == 64:
                        h = cb // nh
                        first = (cb % nh == 0) and s == 0
                        last = (cb % nh == nh - 1) and s == k * k - 1
                        d = dp.tile(dshape, F16, tag=dtag, name="d")
                        emit_d(eng, d[:], xview(kh, kw, ()), col)
                        nc.tensor.matmul(
                            psums[0][32 * h:32 * h + 32, :], lhsT, d[:],
                            start=first, stop=last, tile_position=(0, 32 * h))
                        if cb == nh:  # both halves started: sum-x correction
                            nc.tensor.matmul(
                                psums[0][0:64, :],
                                cst[:, ones_off:ones_off + 64],
                                xview(kh, kw, ()),
                                start=False, stop=False, tile_position=(0, 0))
                    elif co == 32:
                        first = cb == 0 and s == 0
                        last = cb == ncb - 1 and s == k * k - 1
                        for p in range(2):
                            d = dp.tile(dshape, F16, tag=dtag, name="d")
                            emit_d(eng, d[:], xview(kh, kw, (p,)), col)
                            for ii in range(2):
                                c = 2 * p + ii
                                nc.tensor.matmul(
                                    psums[0][32 * c:32 * c + 32, :], lhsT,
                                    d[:, ii, :, :],
                                    start=first, stop=last,
                                    tile_position=(0, 32 * c))
                        if cb == 0:
                            xv = xview(kh, kw, None)
                            for c in range(BL):
                                nc.tensor.matmul(
                                    psums[0][32 * c:32 * c + 32, :],
                                    cst[:, ones_off:ones_off + 32],
                                    xv[:, c, :, :],
                                    start=False, stop=False,
                                    tile_position=(0, 32 * c))
                    else:  # co == 16
                        pass  # handled in the pair loop below
                return psums

            def conv_tail(meta, psums, st, nst):
                if meta["co"] != 16:
                    stats_emit(meta, psums, st)
                st = st_finalize(meta, st)
                sout, offs = allgather([(st, nst)])
                rr = gather_reduce(meta, sout, offs[0])
                evacuate(meta, psums, rr)

            # ---------------- stem ----------------
            with nc.named_scope("stem"):
                pt = pp.tile([27, BL, 32, 32], F16, tag="pt")
                for s in range(9):
                    kh, kw = divmod(s, 3)
                    nc.sync.dma_start(pt[3 * s:3 * s + 3],
                                      xp_d[:, :, kh:kh + 32, kw:kw + 32])
                m_stem = conv_meta(16, 16, 32, 1, 3)
                m_stem.update(outb="X0", evac="relu", name="stem")
                ps_stem = psum_alloc(m_stem, "stem")
                st, nst = st_alloc(m_stem)
                for p in range(2):
                    for ii in range(2):
                        j = 2 * p + ii
                        for b in range(2):
                            rg = 32 * (2 * ii + b)
                            nc.tensor.matmul(
                                ps_stem[p][rg:rg + 16, :], stemw[:],
                                pt[:, j, 16 * b:16 * b + 16, :],
                                start=True, stop=True, tile_position=(0, rg))
                    emit_bank_stats16(st, ps_stem[p], p)
                st = st_finalize(m_stem, st)
                sout, offs = allgather([(st, nst)])
                rr = gather_reduce(m_stem, sout, offs[0])
                evacuate(m_stem, ps_stem, rr)
                if debug:
                    nc.sync.dma_start(dbg_d["stem"][:], bufs["X0"][0:16])

            # ---------------- adder conv layers ----------------
            i = 0
            while i < len(SCHED):
                meta = SCHED[i]
                if meta.get("grp"):  # merged transition pair (tc1 + td)
                    meta2 = SCHED[i + 1]
                    with nc.named_scope(meta["name"]):
                        planes = make_planes(meta)
                        ps1 = adder_conv(meta, planes)
                    with nc.named_scope(meta2["name"]):
                        ps2 = adder_conv(meta2, planes)
                        st1, n1 = st_alloc(meta)
                        st2, n2 = st_alloc(meta2)
                        stats_emit(meta, ps1, st1)
                        stats_emit(meta2, ps2, st2)
                        st1 = st_finalize(meta, st1)
                        st2 = st_finalize(meta2, st2)
                        sout, offs = allgather([(st1, n1), (st2, n2)])
                        rr1 = gather_reduce(meta, sout, offs[0])
                        rr2 = gather_reduce(meta2, sout, offs[1])
                        evacuate(meta, ps1, rr1)
                        evacuate(meta2, ps2, rr2)
                    i += 2
                else:
                    with nc.named_scope(meta["name"]):
                        st, nst = st_alloc(meta)
                        ps = adder_conv(meta, stats_st=st)
                        conv_tail(meta, ps, st, nst)
                    i += 1

            # ---------------- avgpool + fc + final bn ----------------
            with nc.named_scope("fc"):
                zf = bufs[SCHED[-1]["outb"]]
                pooled = sp.tile([64, BL], F32, tag="pool", name="pooled")
                junkp = dp.tile([64, 64], F16, tag="junkp", name="junkp")
                for b in range(BL):
                    nc.scalar.activation(junkp[:], zf[0:64, b, 1:9, 1:9],
                                         AF.Identity,
                                         accum_out=pooled[:, b:b + 1])
                ps_fc = psp.tile([10, BL], F32, tag="ps", name="ps_fc")
                nc.tensor.matmul(ps_fc[:, :], fcw[:], pooled[:], start=True, stop=True)
                st = sp.tile([10, 2], F32, tag="stfc", name="st_fc")
                junk = dp.tile([10, BL], F16, tag="junkfc", name="junk_fc")
                nc.scalar.activation(junk[:], ps_fc[:], AF.Identity,
                                     accum_out=st[:, 0:1])
                nc.scalar.activation(junk[:], ps_fc[:], AF.Square,
                                     accum_out=st[:, 1:2])
                sout, offs = allgather([(st, 20)])
                gst = sp.tile([10, 8, 2], F32, tag="gstfc", name="gst_fc")
                sv = sout[:, 0:20].rearrange("r (i k) -> i r k", i=10, k=2)
                nc.sync.dma_start(gst[:], sv)
                red = sp.tile([10, 2], F32, tag="redfc", name="red_fc")
                nc.vector.tensor_reduce(
                    red[:], gst[:, :, :].rearrange("p r k -> p k r"),
                    mybir.AxisListType.X, A.add)
                rr = sp.tile([10, 2], F32, tag="rrfc", name="rr_fc")
                bn_finish(red, GB, rr, 10)
                osb = sp.tile([10, BL], F32, tag="osb", name="osb")
                nc.scalar.activation(osb[:], ps_fc[:], AF.Identity,
                                     bias=rr[:, 1:2], scale=rr[:, 0:1])
                nc.sync.dma_start(out_d[:], osb[:])

    nc.compile()
    return nc


def get_nc(debug=False):
    key = f"nc{debug}"
    if key not in _CACHE:
        _CACHE[key] = build(debug)
    return _CACHE[key]


# --------------------------------------------------------------------------
# entry point
# --------------------------------------------------------------------------
def kernel(**inputs):
    from concourse.bass_utils import run_bass_kernel_spmd

    x = inputs["x"]  # [32, 3, 32, 32] f32
    wall, cst, stemw, fcw = pack_host(inputs)
    xpad = np.zeros((CORES, 3, BL, 34, 34), np.float16)
    xs = x.reshape(CORES, BL, 3, 32, 32).transpose(0, 2, 1, 3, 4)
    xpad[:, :, :, 1:33, 1:33] = xs.astype(np.float16)

    nc = get_nc()
    in_maps = [{"xp": xpad[i], "wall": wall, "cst": cst,
                "stemw": stemw, "fcw": fcw} for i in range(CORES)]
    res = run_bass_kernel_spmd(nc, in_maps, list(range(CORES)))
    out = np.concatenate([r["out"].T for r in res.results], axis=0)
    return out.astype(np.float32)


# revision 30
# speedup vs baseline: 3.8327x; 1.0150x over previous
"""AdderNet (ResNet20-style, L1-distance convs) on 8 TRN2 NeuronCores.

Self-contained: kernel(**inputs) takes the full unsharded inputs and returns
the full [32, 10] float32 output. Data-parallel over the batch (4 images per
core); BatchNorm batch stats made exact via a per-conv AllGather of
(sum, sumsq) + local reduce.

v2 design (vs baseline):
  - D = |x - w| in ONE DVE op: tensor_scalar(subtract, abs_max 0)
  - PE column-tiling: psum rows 32*j hold different images/chunks, matmuls
    issued to 4 (or 2) distinct 32-column array groups run concurrently
  - stride-2 convs read from stride-1 "parity planes" (precompacted)
  - AllGather (floor ~5us) instead of AllReduce (~10us); the partition
    re-gather happens for free in the return DMA's access pattern
  - replication DMAs spread across engine queues, per-image granularity
"""

import numpy as np

CORES = 8
BL = 4          # local batch per core
EPS = 1e-5
GB = CORES * BL  # global batch

# per-conv D-op engine split: name -> n_act sub-ops routed to ACT (rest DVE).
# GpSimd is never used for tensor ops (measured ~40x slower + SBUF contention).
ENG_SPLIT = {}
for _b in range(3):
    ENG_SPLIT[f"l1b{_b}c1"] = 10
    ENG_SPLIT[f"l1b{_b}c2"] = 8
ENG_SPLIT["l2tc1"] = 24
ENG_SPLIT["l2td"] = 0
for _n in ("l2b0c1", "l2b1c1"):
    ENG_SPLIT[_n] = 26
for _n in ("l2tc2", "l2b0c2", "l2b1c2"):
    ENG_SPLIT[_n] = 22
ENG_SPLIT["l3tc1"] = 32
ENG_SPLIT["l3td"] = 0
for _n in ("l3b0c1", "l3b1c1"):
    ENG_SPLIT[_n] = 52
for _n in ("l3tc2", "l3b0c2", "l3b1c2"):
    ENG_SPLIT[_n] = 44


# --------------------------------------------------------------------------
# network schedule
# --------------------------------------------------------------------------
# cst variants: (ci, g, co). Mblk = min(co, 32).
CST_VARIANTS = [(16, 8, 16), (16, 8, 32), (32, 4, 32), (32, 4, 64), (64, 2, 64)]


def cst_layout():
    off = {}
    ones_off = {}
    o = 0
    for (ci, g, co) in CST_VARIANTS:
        mblk = min(co, 32)
        off[(ci, g, co)] = o
        o += (co // g) * mblk
        ones_off[(ci, g, co)] = o
        o += co
    return off, ones_off, o


def conv_meta(ci, co, hin, stride, k):
    g = 128 // ci
    ncb = co // g
    hout = hin // stride
    bl = BL * hout * hout
    idx = next(i for i, v in enumerate(CST_VARIANTS) if v == (ci, g, co))
    return dict(ci=ci, co=co, g=g, ncb=ncb, k=k, stride=stride,
                hin=hin, hout=hout, bl=bl, cst=idx, mblk=min(co, 32))


def make_schedule():
    convs = []

    def add(name, wsrc, ci, co, hin, stride, k, **roles):
        m = conv_meta(ci, co, hin, stride, k)
        m.update(name=name, wsrc=wsrc, **roles)
        convs.append(m)

    rot = [("X0", "X1", "X2"), ("X2", "X0", "X1"), ("X1", "X2", "X0")]
    for b in range(3):
        i, mid, o = rot[b]
        add(f"l1b{b}c1", ("l1_w", 2 * b), 16, 16, 32, 1, 3, inb=i, outb=mid, evac="relu")
        add(f"l1b{b}c2", ("l1_w", 2 * b + 1), 16, 16, 32, 1, 3, inb=mid, outb=o,
            evac="res", idb=i, idkind="pad")
    add("l2tc1", ("l2_w0",), 16, 32, 32, 2, 3, inb="X0", outb="Y0", evac="relu",
        grp="g2")
    add("l2td", ("l2_down",), 16, 32, 32, 2, 1, inb="X0", outb="ID2", evac="down",
        grp="g2")
    add("l2tc2", ("l2_ws", 0), 32, 32, 16, 1, 3, inb="Y0", outb="Y1", evac="res",
        idb="ID2", idkind="dense")
    rot2 = [("Y1", "Y2", "Y0"), ("Y0", "Y2", "Y1")]
    for b in range(2):
        i, mid, o = rot2[b]
        add(f"l2b{b}c1", ("l2_ws", 1 + 2 * b), 32, 32, 16, 1, 3, inb=i, outb=mid, evac="relu")
        add(f"l2b{b}c2", ("l2_ws", 2 + 2 * b), 32, 32, 16, 1, 3, inb=mid, outb=o,
            evac="res", idb=i, idkind="pad")
    add("l3tc1", ("l3_w0",), 32, 64, 16, 2, 3, inb="Y1", outb="Z0", evac="relu",
        grp="g3")
    add("l3td", ("l3_down",), 32, 64, 16, 2, 1, inb="Y1", outb="ID3", evac="down",
        grp="g3")
    add("l3tc2", ("l3_ws", 0), 64, 64, 8, 1, 3, inb="Z0", outb="Z1", evac="res",
        idb="ID3", idkind="dense")
    rot3 = [("Z1", "Z2", "Z0"), ("Z0", "Z2", "Z1")]
    for b in range(2):
        i, mid, o = rot3[b]
        add(f"l3b{b}c1", ("l3_ws", 1 + 2 * b), 64, 64, 8, 1, 3, inb=i, outb=mid, evac="relu")
        add(f"l3b{b}c2", ("l3_ws", 2 + 2 * b), 64, 64, 8, 1, 3, inb=mid, outb=o,
            evac="res", idb=i, idkind="pad")
    return convs


S_ORDER3 = [0, 2, 3, 5, 6, 8, 1, 4, 7]  # kw==1 last


def d_ops(meta):
    """Yield (cb, s) in emission order. kw==1 shifts come last within each cb
    (they read the shifted shadow buffer, written after replication). For
    co=64, interleave the two halves so consecutive matmuls target
    alternating PE column groups."""
    ncb, k = meta["ncb"], meta["k"]
    s_order = S_ORDER3 if k == 3 else [0]
    if meta["co"] == 64:
        nh = ncb // 2
        for q in range(nh):
            for s in s_order:
                yield q, s
                yield nh + q, s
    else:
        for cb in range(ncb):
            for s in s_order:
                yield cb, s


def d_engine_map(meta):
    """op index (position in d_ops order) -> 'V'/'A'."""
    n_act = ENG_SPLIT.get(meta["name"], 0)
    ops = list(d_ops(meta))
    nsub = {16: BL // 2, 32: BL // 2, 64: 1}[meta["co"]]
    n_act_ops = n_act // nsub
    eng = {}
    if n_act_ops > 0:
        stride = max(1, len(ops) // n_act_ops)
        left = n_act_ops
        for i in range(len(ops)):
            if i % stride == 0 and left > 0:
                eng[i] = "A"
                left -= 1
            else:
                eng[i] = "V"
    else:
        eng = {i: "V" for i in range(len(ops))}
    return ops, eng, nsub


SCHED = make_schedule()
NWALL = sum(c["ncb"] * c["k"] * c["k"] for c in SCHED)
CST_OFF, CST_ONES, NCST = cst_layout()


# --------------------------------------------------------------------------
# host-side packing
# --------------------------------------------------------------------------
def get_w(inputs, wsrc):
    a = inputs[wsrc[0]]
    if len(wsrc) > 1:
        a = a[wsrc[1]]
    return a  # [co, ci, k, k]


def pack_host(inputs):
    wall = np.zeros((128, NWALL), np.float32)
    col = 0
    for m in SCHED:
        w = get_w(inputs, m["wsrc"])
        ci, g, k = m["ci"], m["g"], m["k"]
        for cb, s in d_ops(m):
            kh, kw = divmod(s, k)
            for gg in range(g):
                co = cb * g + gg
                wall[gg * ci:(gg + 1) * ci, col] = w[co, :, kh, kw]
            col += 1
    assert col == NWALL

    cst = np.zeros((128, NCST), np.float16)
    for (ci, g, co) in CST_VARIANTS:
        off = CST_OFF[(ci, g, co)]
        mblk = min(co, 32)
        ncb = co // g
        nper = mblk // g  # blocks per half-window
        for cb in range(ncb):
            q = cb % nper
            for gg in range(g):
                cst[gg * ci:(gg + 1) * ci, off + cb * mblk + q * g + gg] = -2.0
        oo = CST_ONES[(ci, g, co)]
        cst[:, oo:oo + co] = 1.0 / g

    stemw = inputs["conv1_w"].transpose(2, 3, 1, 0).reshape(27, 16).astype(np.float16)
    fcw = (inputs["fc_w"][:, :, 0, 0].T / 64.0).astype(np.float32)  # [64, 10]
    return wall, cst, stemw, fcw


# --------------------------------------------------------------------------
# graph builder
# --------------------------------------------------------------------------
_CACHE = {}


def build(debug=False):
    from concourse import bacc, mybir, tile

    F16, F32 = mybir.dt.float16, mybir.dt.float32
    A = mybir.AluOpType
    AF = mybir.ActivationFunctionType
    AX = mybir.AxisListType

    nc = bacc.Bacc("TRN2", target_bir_lowering=False, debug=False,
                   num_devices=CORES)
    xp_d = nc.dram_tensor("xp", [3, BL, 34, 34], F16, kind="ExternalInput")
    wall_d = nc.dram_tensor("wall", [128, NWALL], F32, kind="ExternalInput")
    cst_d = nc.dram_tensor("cst", [128, NCST], F16, kind="ExternalInput")
    stemw_d = nc.dram_tensor("stemw", [27, 16], F16, kind="ExternalInput")
    fcw_d = nc.dram_tensor("fcw", [64, 10], F32, kind="ExternalInput")
    out_d = nc.dram_tensor("out", [10, BL], F32, kind="ExternalOutput")
    dbg_d = {}
    if debug:
        for m in SCHED:
            shp = ([m["co"], BL, m["hout"] + 2, m["hout"] + 2]
                   if m["evac"] != "down" else [m["co"], BL, m["hout"], m["hout"]])
            dbg_d[m["name"]] = nc.dram_tensor(f'dbg_{m["name"]}', shp,
                                              F16, kind="ExternalOutput")
        dbg_d["stem"] = nc.dram_tensor("dbg_stem", [16, BL, 34, 34],
                                       F16, kind="ExternalOutput")

    with tile.TileContext(nc) as tc:
        import contextlib
        with contextlib.ExitStack() as ctx:
            pp = ctx.enter_context(tc.tile_pool(name="persist", bufs=1))
            dp = ctx.enter_context(tc.tile_pool(name="dtiles", bufs=6))
            sp = ctx.enter_context(tc.tile_pool(name="small", bufs=8))
            ppl = ctx.enter_context(tc.tile_pool(name="planes", bufs=6))
            psp = ctx.enter_context(tc.tile_pool(name="psum", bufs=8, space="PSUM"))
            drp = ctx.enter_context(tc.tile_pool(name="dram", bufs=4, space="DRAM"))

            wall = pp.tile([128, NWALL], F32, tag="wall")
            nwall = pp.tile([128, NWALL], F32, tag="nwall")
            cst = pp.tile([128, NCST], F16, tag="cst")
            stemw = pp.tile([27, 16], F16, tag="stemw")
            fcw = pp.tile([64, 10], F32, tag="fcw")
            epst = pp.tile([128, 1], F32, tag="epst")
            nc.sync.dma_start(wall[:], wall_d[:])
            nc.sync.dma_start(cst[:], cst_d[:])
            nc.sync.dma_start(stemw[:], stemw_d[:])
            nc.sync.dma_start(fcw[:], fcw_d[:])
            nc.vector.memset(epst[:], EPS)
            nc.vector.tensor_scalar(nwall[:], wall[:], -1.0, None, A.mult)

            # activation buffers (persistent, zeroed once => borders stay 0)
            bufs = {}
            for nm in ("X0", "X1", "X2"):
                bufs[nm] = pp.tile([128, BL, 34, 34], F16, name=nm, tag=nm)
            for nm in ("Y0", "Y1", "Y2"):
                bufs[nm] = pp.tile([128, BL, 18, 18], F16, name=nm, tag=nm)
            for nm in ("Z0", "Z1", "Z2"):
                bufs[nm] = pp.tile([128, BL, 10, 10], F16, name=nm, tag=nm)
            bufs["ID2"] = pp.tile([128, BL, 16, 16], F16, name="ID2", tag="ID2")
            bufs["ID3"] = pp.tile([128, BL, 8, 8], F16, name="ID3", tag="ID3")
            # shifted shadow copies (one column left) so kw==1 D-reads stay
            # 4-byte aligned for the DVE 4x mode
            for nm in ("X0", "X1", "X2"):
                bufs[nm + "s"] = pp.tile([128, BL, 34, 33], F16, name=nm + "s",
                                         tag=nm + "s")
            for nm in ("Y0", "Y1", "Y2"):
                bufs[nm + "s"] = pp.tile([128, BL, 18, 17], F16, name=nm + "s",
                                         tag=nm + "s")
            for nm in ("Z0", "Z1", "Z2"):
                bufs[nm + "s"] = pp.tile([128, BL, 10, 9], F16, name=nm + "s",
                                         tag=nm + "s")
            for nm in ("X0", "X1", "X2", "Y0", "Y1", "Y2", "Z0", "Z2", "Z1"):
                nc.vector.memset(bufs[nm][:], 0.0)
                nc.vector.memset(bufs[nm + "s"][:], 0.0)

            # round-robin DMA queue picker for replication copies
            rq_engines = None
            rq_i = [0]

            def rqueue():
                e = rq_engines[rq_i[0] % len(rq_engines)]
                rq_i[0] += 1
                return e
            rq_engines = [nc.sync, nc.gpsimd]

            # ---------------- BN helpers ----------------
            def bn_finish(gred, n, rr, rows):
                """gred: [rows, 2] (S1, S2) global sums tile. rr: [rows, 2]
                out (r, -m*r). ACT-heavy to minimize engine switches."""
                mt = sp.tile([rows, 4], F32, tag="bnm", name="bnm")
                nc.scalar.activation(rr[:, 2:3], gred[:, 0:1], AF.Identity,
                                     scale=1.0 / n)                  # m
                nc.scalar.activation(mt[:, 1:2], rr[:, 2:3], AF.Square)  # m^2
                nc.vector.tensor_scalar(mt[:, 2:3], gred[:, 1:2], 1.0 / n,
                                        mt[:, 1:2], A.mult, A.subtract)
                nc.scalar.activation(mt[:, 3:4], mt[:, 2:3], AF.Sqrt,
                                     bias=epst[0:rows, 0:1])
                nc.vector.reciprocal(rr[:, 0:1], mt[:, 3:4])
                nc.vector.tensor_scalar(rr[:, 1:2], rr[:, 2:3], -1.0,
                                        rr[:, 0:1], A.mult, A.mult)

            def allgather(st_tiles):
                """st_tiles: list of (tile, nelem_f32). Returns DRAM agout tile
                + per-input offset list. agout layout: [8 ranks, sum(nelem)]."""
                tot = sum(n for _, n in st_tiles)
                sin = drp.tile([tot], F32, tag="agi", name="agi")
                offs = []
                o = 0
                for t, n in st_tiles:
                    nc.sync.dma_start(sin[o:o + n], t[:])
                    offs.append(o)
                    o += n
                sout = drp.tile([CORES, tot], F32, tag="ago", name="ago")
                nc.gpsimd.collective_compute(
                    "AllGather", A.bypass,
                    replica_groups=[list(range(CORES))],
                    ins=[sin.opt()], outs=[sout.opt()],
                )
                return sout, offs

            # ---------------- per-layout helpers ----------------
            # layouts keyed by co: how psum / stats / evac are organized.
            def psum_alloc(meta, name):
                co = meta["co"]
                if co == 16:
                    return [psp.tile([128, 512], F32, tag="ps", name=f"{name}_b{b}")
                            for b in range(2)]
                if co == 32:
                    return [psp.tile([128, 256], F32, tag="ps", name=f"{name}_b0")]
                return [psp.tile([64, 256], F32, tag="ps", name=f"{name}_b0")]

            def emit_bank_stats16(st, psum, b):
                jk = dp.tile([128, 512], F16, tag="junk", name="junk")
                nc.vector.tensor_scalar(jk[:], psum[:], 0.0, None,
                                        A.add, A.add,
                                        accum_out=st[:, 2 * b:2 * b + 1])
                nc.scalar.activation(jk[:], psum[:], AF.Square,
                                     accum_out=st[:, 2 * b + 1:2 * b + 2])

            def stats_emit(meta, psums, st):
                co = meta["co"]
                if co == 16:
                    for b in range(2):
                        emit_bank_stats16(st, psums[b], b)
                elif co == 32:
                    jk = dp.tile([128, 256], F16, tag="junk", name="junk")
                    nc.vector.tensor_scalar(jk[:], psums[0][:], 0.0, None,
                                            A.add, A.add, accum_out=st[:, 0:1])
                    nc.scalar.activation(jk[:], psums[0][:], AF.Square,
                                         accum_out=st[:, 1:2])
                else:
                    jk = dp.tile([64, 256], F16, tag="junk64", name="junk")
                    nc.vector.tensor_scalar(jk[:], psums[0][:], 0.0, None,
                                            A.add, A.add, accum_out=st[:, 0:1])
                    nc.scalar.activation(jk[:], psums[0][:], AF.Square,
                                         accum_out=st[:, 1:2])

            def st_alloc(meta):
                co = meta["co"]
                if co == 16:
                    return sp.tile([128, 4], F32, tag="st4", name=f"st_{meta['name']}"), 384
                if co == 32:
                    return sp.tile([128, 3], F32, tag="st", name=f"st_{meta['name']}"), 384
                return sp.tile([64, 3], F32, tag="st64", name=f"st_{meta['name']}"), 192

            def st_finalize(meta, st):
                """For co16: combine the two banks' partial stats -> [128, 3]."""
                if meta["co"] != 16:
                    return st
                st2 = sp.tile([128, 3], F32, tag="st", name="st2")
                nc.vector.tensor_tensor(st2[:, 0:2], st[:, 0:2], st[:, 2:4], A.add)
                return st2

            def gather_reduce(meta, sout, off):
                """Gather the AG output into per-channel layout + reduce + bn.
                Blob layout per rank: flat st2 [rows, 2] (row-major).
                Returns rr tile ([128,2] for co<=32 replicated, [64,2] co=64)."""
                co = meta["co"]
                n = GB * meta["hout"] * meta["hout"]
                if co == 64:
                    gst = sp.tile([64, 8, 2], F32, tag="gst64", name="gst")
                    sv = sout[:, off:off + 192].rearrange(
                        "r (i k) -> i r k", i=64, k=3)[:, :, 0:2]
                    nc.sync.dma_start(gst[:], sv)
                    red = sp.tile([64, 2], F32, tag="red64", name="red")
                    nc.vector.tensor_reduce(
                        red[:], gst[:, :, :].rearrange("p r k -> p k r"),
                        AX.X, A.add)
                    rr = sp.tile([64, 3], F32, tag="rr64", name="rr")
                    bn_finish(red, n, rr, 64)
                    return rr
                nch = co  # channels live at rows 32j+0:co
                gst = sp.tile([nch, 4, 8, 2], F32, tag="gst", name="gst")
                for j in range(4):
                    sv = sout[:, off + 96 * j:off + 96 * j + 3 * nch].rearrange(
                        "r (i k) -> i r k", i=nch, k=3)[:, :, 0:2]
                    (nc.sync if j % 2 == 0 else nc.gpsimd).dma_start(
                        gst[:, j, :, :], sv)
                red = sp.tile([nch, 2], F32, tag="red", name="red")
                nc.vector.tensor_reduce(
                    red[:], gst[:, :, :, :].rearrange("p j r k -> p k (j r)"),
                    AX.X, A.add)
                rr = sp.tile([128, 3], F32, tag="rr", name="rr")
                bn_finish(red, n, rr[0:nch, :], nch)
                for t, eng in ((1, nc.scalar), (2, nc.gpsimd), (3, nc.sync)):
                    eng.dma_start(rr[32 * t:32 * t + nch, :], rr[0:nch, :])
                return rr

            def evacuate(meta, psums, rr):
                """psum -> xout (+ per-image replication)."""
                co, hout = meta["co"], meta["hout"]
                xout = bufs[meta["outb"]]
                kind = meta["evac"]
                idt = bufs[meta["idb"]] if kind == "res" else None
                if co == 16:
                    for j in range(BL):
                        for b in range(2):
                            rg = 32 * (2 * (j % 2) + b)
                            ps = psums[j // 2][rg:rg + 16, :]
                            ov = xout[rg:rg + 16, j,
                                      1 + 16 * b:17 + 16 * b, 1:33]
                            rrs = rr[rg:rg + 16, :]
                            if kind == "res":
                                idv = idt[rg:rg + 16, j,
                                          1 + 16 * b:17 + 16 * b, 1:33]
                                t = dp.tile([128, 512], F16, tag="tres", name="tres")
                                ts = t[rg:rg + 16, :]
                                nc.vector.scalar_tensor_tensor(
                                    ts, ps, rrs[:, 0:1], idv, A.mult, A.add)
                                if (j + b) % 2 == 0:
                                    nc.scalar.activation(ov, ts, AF.Relu,
                                                         bias=rrs[:, 1:2])
                                else:
                                    nc.vector.tensor_scalar(
                                        ov, ts, rrs[:, 1:2], 0.0, A.add, A.max)
                            else:
                                if (j + b) % 2 == 0:
                                    nc.scalar.activation(ov, ps, AF.Relu,
                                                         bias=rrs[:, 1:2],
                                                         scale=rrs[:, 0:1])
                                else:
                                    # relu(bn(u)) = (max(u,m)-m)*r on DVE
                                    t1 = dp.tile([128, 512], F16, tag="tres",
                                                 name="t1")
                                    nc.vector.tensor_scalar(
                                        t1[rg:rg + 16, :], ps, rrs[:, 2:3],
                                        rrs[:, 2:3], A.max, A.subtract)
                                    nc.vector.tensor_scalar(
                                        ov, t1[rg:rg + 16, :], rrs[:, 0:1],
                                        None, A.mult)
                        # assemble + replicate image j to all 8 groups:
                        # halves evacuated to row-groups rb (top) and rb+32
                        # (bottom) -> cross-copy, double, then 64->64
                        rb = 64 * (j % 2)
                        rqueue().dma_start(xout[rb:rb + 16, j, 17:33, :],
                                           xout[rb + 32:rb + 48, j, 17:33, :])
                        rqueue().dma_start(xout[rb + 32:rb + 48, j, 0:17, :],
                                           xout[rb:rb + 16, j, 0:17, :])
                        rqueue().dma_start(xout[rb + 16:rb + 32, j, :, :],
                                           xout[rb:rb + 16, j, :, :])
                        rqueue().dma_start(xout[rb + 48:rb + 64, j, :, :],
                                           xout[rb + 32:rb + 48, j, :, :])
                        ro = (rb + 64) % 128
                        rqueue().dma_start(xout[ro:ro + 64, j, :, :],
                                           xout[rb:rb + 64, j, :, :])
                        xsh = bufs.get(meta["outb"] + "s")
                        if xsh is not None:
                            rqueue().dma_start(xsh[:, j, :, 0:33],
                                               xout[:, j, :, 1:34])
                elif co == 32:
                    hp = hout + 2
                    for c in range(BL):
                        ps = psums[0][32 * c:32 * c + 32, :]
                        rrs = rr[32 * c:32 * c + 32, :]
                        if kind == "down":
                            ov = bufs["ID2"][32 * c:32 * c + 32, c, :, :]
                            if c % 2 == 0:
                                nc.scalar.activation(ov, ps, AF.Identity,
                                                     bias=rrs[:, 1:2],
                                                     scale=rrs[:, 0:1])
                            else:
                                nc.vector.tensor_scalar(
                                    ov, ps, rrs[:, 0:1], rrs[:, 1:2],
                                    A.mult, A.add)
                            src = bufs["ID2"][32 * c:32 * c + 32, c, :, :]
                            dstbuf = bufs["ID2"]
                            sh = [hout, hout]
                        else:
                            ov = xout[32 * c:32 * c + 32, c, 1:1 + hout, 1:1 + hout]
                            if kind == "res":
                                if meta["idkind"] == "pad":
                                    idv = idt[32 * c:32 * c + 32, c,
                                              1:1 + hout, 1:1 + hout]
                                else:
                                    idv = idt[32 * c:32 * c + 32, c, :, :]
                                t = dp.tile([128, 256], F16, tag="tres32", name="tres")
                                ts = t[32 * c:32 * c + 32, :]
                                nc.vector.scalar_tensor_tensor(
                                    ts, ps, rrs[:, 0:1], idv, A.mult, A.add)
                                if c % 2 == 0:
                                    nc.scalar.activation(ov, ts, AF.Relu,
                                                         bias=rrs[:, 1:2])
                                else:
                                    nc.vector.tensor_scalar(
                                        ov, ts, rrs[:, 1:2], 0.0, A.add, A.max)
                            else:
                                if c % 2 == 0:
                                    nc.scalar.activation(ov, ps, AF.Relu,
                                                         bias=rrs[:, 1:2],
                                                         scale=rrs[:, 0:1])
                                else:
                                    t1 = dp.tile([128, 256], F16, tag="tres32",
                                                 name="t1")
                                    nc.vector.tensor_scalar(
                                        t1[32 * c:32 * c + 32, :], ps,
                                        rrs[:, 2:3], rrs[:, 2:3],
                                        A.max, A.subtract)
                                    nc.vector.tensor_scalar(
                                        ov, t1[32 * c:32 * c + 32, :],
                                        rrs[:, 0:1], None, A.mult)
                            src = xout[32 * c:32 * c + 32, c, :, :]
                            dstbuf = xout
                            sh = [hp, hp]
                        rb = 32 * c
                        rp = rb ^ 32
                        rqueue().dma_start(dstbuf[rp:rp + 32, c, :, :], src)
                        rh = rb // 64 * 64
                        ro = rh ^ 64
                        rqueue().dma_start(dstbuf[ro:ro + 64, c, :, :],
                                           dstbuf[rh:rh + 64, c, :, :])
                        if kind != "down":
                            xsh = bufs.get(meta["outb"] + "s")
                            if xsh is not None:
                                rqueue().dma_start(xsh[:, c, :, 0:17],
                                                   xout[:, c, :, 1:18])
                else:  # co == 64
                    ps = psums[0][:, :]
                    if kind == "down":
                        ov = bufs["ID3"][0:64, :, :, :]
                        nc.scalar.activation(ov, ps, AF.Identity,
                                             bias=rr[:, 1:2], scale=rr[:, 0:1])
                        nc.sync.dma_start(bufs["ID3"][64:128, :, :, :],
                                          bufs["ID3"][0:64, :, :, :])
                    else:
                        nim = hout * hout
                        if kind == "res":
                            t = dp.tile([64, 256], F16, tag="tres64", name="tres")
                            for b in range(BL):
                                if meta["idkind"] == "pad":
                                    idv = idt[0:64, b, 1:1 + hout, 1:1 + hout]
                                else:
                                    idv = idt[0:64, b, :, :]
                                nc.vector.scalar_tensor_tensor(
                                    t[:, nim * b:nim * b + nim],
                                    psums[0][:, nim * b:nim * b + nim],
                                    rr[:, 0:1], idv, A.mult, A.add)
                            for b in range(BL):
                                nc.scalar.activation(
                                    xout[0:64, b, 1:1 + hout, 1:1 + hout],
                                    t[:, nim * b:nim * b + nim],
                                    AF.Relu, bias=rr[:, 1:2])
                        else:
                            for b in range(BL):
                                nc.scalar.activation(
                                    xout[0:64, b, 1:1 + hout, 1:1 + hout],
                                    psums[0][:, nim * b:nim * b + nim],
                                    AF.Relu, bias=rr[:, 1:2], scale=rr[:, 0:1])
                        nc.sync.dma_start(xout[64:128, :, :, :],
                                          xout[0:64, :, :, :])
                        xsh = bufs.get(meta["outb"] + "s")
                        if xsh is not None:
                            rqueue().dma_start(xsh[:, :, :, 0:9],
                                               xout[:, :, :, 1:10])
                if debug and meta["name"] in dbg_d:
                    if kind == "down":
                        db = bufs["ID2"] if co == 32 else bufs["ID3"]
                        nc.sync.dma_start(dbg_d[meta["name"]][:], db[0:co])
                    else:
                        nc.sync.dma_start(dbg_d[meta["name"]][:], xout[0:co])

            # ---------------- parity planes for stride-2 convs ----------------
            def make_planes(meta):
                """Precompact stride-2 input into 4 stride-1 parity planes."""
                xin = bufs[meta["inb"]]
                hin = meta["hin"]          # 32 or 16
                hh = hin // 2 + 1          # 17 or 9
                wpl = hh + 1               # even width
                planes = {}
                engs = [nc.vector, nc.gpsimd, nc.vector, nc.gpsimd]
                i = 0
                for pr in (0, 1):
                    for pc in (0, 1):
                        pl = ppl.tile([128, BL, hh, wpl], F16,
                                      tag=f"pl{hin}", name=f"pl{pr}{pc}")
                        src = xin[:, :, pr:pr + 2 * hh - 1:2, pc:pc + 2 * hh - 1:2]
                        engs[i % 4].tensor_scalar(pl[:, :, :, 0:hh], src, 0.0,
                                                  None, A.add)
                        i += 1
                        planes[(pr, pc)] = pl
                return planes

            # ---------------- adder conv core ----------------
            wall_col = [0]

            def adder_conv(meta, planes=None, stats_st=None):
                ci, co, g, ncb, k = meta["ci"], meta["co"], meta["g"], meta["ncb"], meta["k"]
                hout, stride = meta["hout"], meta["stride"]
                mblk = meta["mblk"]
                xin = bufs[meta["inb"]]
                coff = CST_OFF[CST_VARIANTS[meta["cst"]]]
                ones_off = CST_ONES[CST_VARIANTS[meta["cst"]]]
                psums = psum_alloc(meta, meta["name"])
                ops, engmap, nsub = d_engine_map(meta)
                ncol = {16: 512, 32: 256, 64: 256}[co]
                nh = ncb // 2 if co == 64 else None

                def xview(kh, kw, sub):
                    if stride == 2:
                        if k == 1:
                            pl, r0, c0 = planes[(1, 1)], 0, 0
                        else:
                            pl = planes[(kh % 2, kw % 2)]
                            r0, c0 = kh // 2, kw // 2
                        if co == 32 and sub is not None:  # per image-pair
                            p, = sub
                            return pl[:, 2 * p:2 * p + 2, r0:r0 + hout, c0:c0 + hout]
                        return pl[:, :, r0:r0 + hout, c0:c0 + hout]
                    if co == 16:
                        j, = sub
                        return xin[:, j, kh:kh + hout, kw:kw + hout]
                    if co == 32:
                        if sub is None:
                            return xin[:, :, kh:kh + hout, kw:kw + hout]
                        p, = sub
                        return xin[:, 2 * p:2 * p + 2, kh:kh + hout, kw:kw + hout]
                    return xin[:, :, kh:kh + hout, kw:kw + hout]

                def emit_d(eng, dv, xv, col):
                    # max-form: D = max(x, w) (DVE/GPS) or relu(x - w) (ACT);
                    # blockdiag(-2) + a sum-x ones matmul recovers -sum|x-w|
                    # up to a per-channel constant absorbed by BN.
                    if eng == "A":
                        nc.scalar.activation(dv, xv, AF.Relu,
                                             bias=nwall[:, col:col + 1])
                    elif eng == "G":
                        nc.gpsimd.tensor_scalar(dv, xv, wall[:, col:col + 1],
                                                None, A.max)
                    else:
                        nc.vector.tensor_scalar(dv, xv, wall[:, col:col + 1],
                                                None, A.max)

                dshape = {16: [128, hout, hout], 32: [128, 2, hout, hout],
                          64: [128, BL, hout, hout]}[co]
                dtag = f"d{co}_{hout}"

                if co == 16:
                    # pair-split: all ops for images {0,1} (bank 0), then
                    # images {2,3} (bank 1); per-bank stats emitted inline so
                    # the AllGather can start while pair 1 computes.
                    colbase = wall_col[0]
                    wall_col[0] += len(ops)
                    for p in range(2):
                        for oi, (cb, s) in enumerate(ops):
                            kh, kw = divmod(s, k)
                            col = colbase + oi
                            eng = engmap[oi]
                            lhsT = cst[:, coff + cb * mblk:coff + (cb + 1) * mblk]
                            first = oi == 0
                            last = oi == len(ops) - 1
                            d = dp.tile(dshape, F16, tag=dtag, name="d")
                            emit_d(eng, d[:], xview(kh, kw, (p,)), col)
                            for ii in range(2):
                                for b in range(2):
                                    rg = 32 * (2 * ii + b)
                                    nc.tensor.matmul(
                                        psums[p][rg:rg + 16, :], lhsT,
                                        d[:, ii, 16 * b:16 * b + 16, :],
                                        start=first, stop=last,
                                        tile_position=(0, rg))
                            if cb == 0:
                                xv = xview(kh, kw, (p,))
                                for ii in range(2):
                                    for b in range(2):
                                        rg = 32 * (2 * ii + b)
                                        nc.tensor.matmul(
                                            psums[p][rg:rg + 16, :],
                                            cst[:, ones_off:ones_off + 16],
                                            xv[:, ii, 16 * b:16 * b + 16, :],
                                            start=False, stop=False,
                                            tile_position=(0, rg))
                        if stats_st is not None:
                            emit_bank_stats16(stats_st, psums[p], p)
                    return psums

                for oi, (cb, s) in enumerate(ops):
                    kh, kw = divmod(s, k)
                    col = wall_col[0]
                    wall_col[0] += 1
                    eng = engmap[oi]
                    lhsT = cst[:, coff + cb * mblk:coff + (cb + 1) * mblk]
                    if co == 64:
                        h = cb // nh
                        first = (cb % nh == 0) and s == 0
                        last = (cb % nh == nh - 1) and s == k * k - 1
                        d = dp.tile(dshape, F16, tag=dtag, name="d")
                        emit_d(eng, d[:], xview(kh, kw, ()), col)
                        nc.tensor.matmul(
                            psums[0][32 * h:32 * h + 32, :], lhsT, d[:],
                            start=first, stop=last, tile_position=(0, 32 * h))
                        if cb == nh:  # both halves started: sum-x correction
                            nc.tensor.matmul(
                                psums[0][0:64, :],
                                cst[:, ones_off:ones_off + 64],
                                xview(kh, kw, ()),
                                start=False, stop=False, tile_position=(0, 0))
                    elif co == 32:
                        first = cb == 0 and s == 0
                        last = cb == ncb - 1 and s == k * k - 1
                        for p in range(2):
                            d = dp.tile(dshape, F16, tag=dtag, name="d")
                            emit_d(eng, d[:], xview(kh, kw, (p,)), col)
                            for ii in range(2):
                                c = 2 * p + ii
                                nc.tensor.matmul(
                                    psums[0][32 * c:32 * c + 32, :], lhsT,
                                    d[:, ii, :, :],
                                    start=first, stop=last,
                                    tile_position=(0, 32 * c))
                        if cb == 0:
                            xv = xview(kh, kw, None)
                            for c in range(BL):
                                nc.tensor.matmul(
                                    psums[0][32 * c:32 * c + 32, :],
                                    cst[:, ones_off:ones_off + 32],
                                    xv[:, c, :, :],
                                    start=False, stop=False,
                                    tile_position=(0, 32 * c))
                    else:  # co == 16
                        pass  # handled in the pair loop below
                return psums

            def conv_tail(meta, psums, st, nst):
                if meta["co"] != 16:
                    stats_emit(meta, psums, st)
                st = st_finalize(meta, st)
                sout, offs = allgather([(st, nst)])
                rr = gather_reduce(meta, sout, offs[0])
                evacuate(meta, psums, rr)

            # ---------------- stem ----------------
            with nc.named_scope("stem"):
                pt = pp.tile([27, BL, 32, 32], F16, tag="pt")
                for s in range(9):
                    kh, kw = divmod(s, 3)
                    nc.sync.dma_start(pt[3 * s:3 * s + 3],
                                      xp_d[:, :, kh:kh + 32, kw:kw + 32])
                m_stem = conv_meta(16, 16, 32, 1, 3)
                m_stem.update(outb="X0", evac="relu", name="stem")
                ps_stem = psum_alloc(m_stem, "stem")
                st, nst = st_alloc(m_stem)
                for p in range(2):
                    for ii in range(2):
                        j = 2 * p + ii
                        for b in range(2):
                            rg = 32 * (2 * ii + b)
                            nc.tensor.matmul(
                                ps_stem[p][rg:rg + 16, :], stemw[:],
                                pt[:, j, 16 * b:16 * b + 16, :],
                                start=True, stop=True, tile_position=(0, rg))
                    emit_bank_stats16(st, ps_stem[p], p)
                st = st_finalize(m_stem, st)
                sout, offs = allgather([(st, nst)])
                rr = gather_reduce(m_stem, sout, offs[0])
                evacuate(m_stem, ps_stem, rr)
                if debug:
                    nc.sync.dma_start(dbg_d["stem"][:], bufs["X0"][0:16])

            # ---------------- adder conv layers ----------------
            i = 0
            while i < len(SCHED):
                meta = SCHED[i]
                if meta.get("grp"):  # merged transition pair (tc1 + td)
                    meta2 = SCHED[i + 1]
                    with nc.named_scope(meta["name"]):
                        planes = make_planes(meta)
                        ps1 = adder_conv(meta, planes)
                    with nc.named_scope(meta2["name"]):
                        ps2 = adder_conv(meta2, planes)
                        st1, n1 = st_alloc(meta)
                        st2, n2 = st_alloc(meta2)
                        stats_emit(meta, ps1, st1)
                        stats_emit(meta2, ps2, st2)
                        st1 = st_finalize(meta, st1)
                        st2 = st_finalize(meta2, st2)
                        sout, offs = allgather([(st1, n1), (st2, n2)])
                        rr1 = gather_reduce(meta, sout, offs[0])
                        rr2 = gather_reduce(meta2, sout, offs[1])
                        evacuate(meta, ps1, rr1)
                        evacuate(meta2, ps2, rr2)
                    i += 2
                else:
                    with nc.named_scope(meta["name"]):
                        st, nst = st_alloc(meta)
                        ps = adder_conv(meta, stats_st=st)
                        conv_tail(meta, ps, st, nst)
                    i += 1

            # ---------------- avgpool + fc + final bn ----------------
            with nc.named_scope("fc"):
                zf = bufs[SCHED[-1]["outb"]]
                pooled = sp.tile([64, BL], F32, tag="pool", name="pooled")
                junkp = dp.tile([64, 64], F16, tag="junkp", name="junkp")
                for b in range(BL):
                    nc.scalar.activation(junkp[:], zf[0:64, b, 1:9, 1:9],
                                         AF.Identity,
                                         accum_out=pooled[:, b:b + 1])
                ps_fc = psp.tile([10, BL], F32, tag="ps", name="ps_fc")
                nc.tensor.matmul(ps_fc[:, :], fcw[:], pooled[:], start=True, stop=True)
                st = sp.tile([10, 2], F32, tag="stfc", name="st_fc")
                junk = dp.tile([10, BL], F16, tag="junkfc", name="junk_fc")
                nc.scalar.activation(junk[:], ps_fc[:], AF.Identity,
                                     accum_out=st[:, 0:1])
                nc.scalar.activation(junk[:], ps_fc[:], AF.Square,
                                     accum_out=st[:, 1:2])
                sout, offs = allgather([(st, 20)])
                gst = sp.tile([10, 8, 2], F32, tag="gstfc", name="gst_fc")
                sv = sout[:, 0:20].rearrange("r (i k) -> i r k", i=10, k=2)
                nc.sync.dma_start(gst[:], sv)
                red = sp.tile([10, 2], F32, tag="redfc", name="red_fc")
                nc.vector.tensor_reduce(
                    red[:], gst[:, :, :].rearrange("p r k -> p k r"),
                    mybir.AxisListType.X, A.add)
                rr = sp.tile([10, 3], F32, tag="rrfc", name="rr_fc")
                bn_finish(red, GB, rr, 10)
                osb = sp.tile([10, BL], F32, tag="osb", name="osb")
                nc.scalar.activation(osb[:], ps_fc[:], AF.Identity,
                                     bias=rr[:, 1:2], scale=rr[:, 0:1])
                nc.sync.dma_start(out_d[:], osb[:])

    nc.compile()
    return nc


def get_nc(debug=False):
    key = f"nc{debug}"
    if key not in _CACHE:
        _CACHE[key] = build(debug)
    return _CACHE[key]


# --------------------------------------------------------------------------
# entry point
# --------------------------------------------------------------------------
def kernel(**inputs):
    from concourse.bass_utils import run_bass_kernel_spmd

    x = inputs["x"]  # [32, 3, 32, 32] f32
    wall, cst, stemw, fcw = pack_host(inputs)
    xpad = np.zeros((CORES, 3, BL, 34, 34), np.float16)
    xs = x.reshape(CORES, BL, 3, 32, 32).transpose(0, 2, 1, 3, 4)
    xpad[:, :, :, 1:33, 1:33] = xs.astype(np.float16)

    nc = get_nc()
    in_maps = [{"xp": xpad[i], "wall": wall, "cst": cst,
                "stemw": stemw, "fcw": fcw} for i in range(CORES)]
    res = run_bass_kernel_spmd(nc, in_maps, list(range(CORES)))
    out = np.concatenate([r["out"].T for r in res.results], axis=0)
    return out.astype(np.float32)


# revision 31
# speedup vs baseline: 3.8681x; 1.0093x over previous
"""AdderNet (ResNet20-style, L1-distance convs) on 8 TRN2 NeuronCores.

Self-contained: kernel(**inputs) takes the full unsharded inputs and returns
the full [32, 10] float32 output. Data-parallel over the batch (4 images per
core); BatchNorm batch stats made exact via a per-conv AllGather of
(sum, sumsq) + local reduce.

v2 design (vs baseline):
  - D = |x - w| in ONE DVE op: tensor_scalar(subtract, abs_max 0)
  - PE column-tiling: psum rows 32*j hold different images/chunks, matmuls
    issued to 4 (or 2) distinct 32-column array groups run concurrently
  - stride-2 convs read from stride-1 "parity planes" (precompacted)
  - AllGather (floor ~5us) instead of AllReduce (~10us); the partition
    re-gather happens for free in the return DMA's access pattern
  - replication DMAs spread across engine queues, per-image granularity
"""

import numpy as np

CORES = 8
BL = 4          # local batch per core
EPS = 1e-5
GB = CORES * BL  # global batch

# per-conv D-op engine split: name -> n_act sub-ops routed to ACT (rest DVE).
# GpSimd is never used for tensor ops (measured ~40x slower + SBUF contention).
ENG_SPLIT = {}
for _b in range(3):
    ENG_SPLIT[f"l1b{_b}c1"] = 10
    ENG_SPLIT[f"l1b{_b}c2"] = 8
ENG_SPLIT["l2tc1"] = 24
ENG_SPLIT["l2td"] = 0
for _n in ("l2b0c1", "l2b1c1"):
    ENG_SPLIT[_n] = 26
for _n in ("l2tc2", "l2b0c2", "l2b1c2"):
    ENG_SPLIT[_n] = 22
ENG_SPLIT["l3tc1"] = 32
ENG_SPLIT["l3td"] = 0
for _n in ("l3b0c1", "l3b1c1"):
    ENG_SPLIT[_n] = 52
for _n in ("l3tc2", "l3b0c2", "l3b1c2"):
    ENG_SPLIT[_n] = 44


# --------------------------------------------------------------------------
# network schedule
# --------------------------------------------------------------------------
# cst variants: (ci, g, co). Mblk = min(co, 32).
CST_VARIANTS = [(16, 8, 16), (16, 8, 32), (32, 4, 32), (32, 4, 64), (64, 2, 64)]


def cst_layout():
    off = {}
    ones_off = {}
    o = 0
    for (ci, g, co) in CST_VARIANTS:
        mblk = min(co, 32)
        off[(ci, g, co)] = o
        o += (co // g) * mblk
        ones_off[(ci, g, co)] = o
        o += co
    return off, ones_off, o


def conv_meta(ci, co, hin, stride, k):
    g = 128 // ci
    ncb = co // g
    hout = hin // stride
    bl = BL * hout * hout
    idx = next(i for i, v in enumerate(CST_VARIANTS) if v == (ci, g, co))
    return dict(ci=ci, co=co, g=g, ncb=ncb, k=k, stride=stride,
                hin=hin, hout=hout, bl=bl, cst=idx, mblk=min(co, 32))


def make_schedule():
    convs = []

    def add(name, wsrc, ci, co, hin, stride, k, **roles):
        m = conv_meta(ci, co, hin, stride, k)
        m.update(name=name, wsrc=wsrc, **roles)
        convs.append(m)

    rot = [("X0", "X1", "X2"), ("X2", "X0", "X1"), ("X1", "X2", "X0")]
    for b in range(3):
        i, mid, o = rot[b]
        add(f"l1b{b}c1", ("l1_w", 2 * b), 16, 16, 32, 1, 3, inb=i, outb=mid, evac="relu")
        add(f"l1b{b}c2", ("l1_w", 2 * b + 1), 16, 16, 32, 1, 3, inb=mid, outb=o,
            evac="res", idb=i, idkind="pad")
    add("l2tc1", ("l2_w0",), 16, 32, 32, 2, 3, inb="X0", outb="Y0", evac="relu",
        grp="g2")
    add("l2td", ("l2_down",), 16, 32, 32, 2, 1, inb="X0", outb="ID2", evac="down",
        grp="g2")
    add("l2tc2", ("l2_ws", 0), 32, 32, 16, 1, 3, inb="Y0", outb="Y1", evac="res",
        idb="ID2", idkind="dense")
    rot2 = [("Y1", "Y2", "Y0"), ("Y0", "Y2", "Y1")]
    for b in range(2):
        i, mid, o = rot2[b]
        add(f"l2b{b}c1", ("l2_ws", 1 + 2 * b), 32, 32, 16, 1, 3, inb=i, outb=mid, evac="relu")
        add(f"l2b{b}c2", ("l2_ws", 2 + 2 * b), 32, 32, 16, 1, 3, inb=mid, outb=o,
            evac="res", idb=i, idkind="pad")
    add("l3tc1", ("l3_w0",), 32, 64, 16, 2, 3, inb="Y1", outb="Z0", evac="relu",
        grp="g3")
    add("l3td", ("l3_down",), 32, 64, 16, 2, 1, inb="Y1", outb="ID3", evac="down",
        grp="g3")
    add("l3tc2", ("l3_ws", 0), 64, 64, 8, 1, 3, inb="Z0", outb="Z1", evac="res",
        idb="ID3", idkind="dense")
    rot3 = [("Z1", "Z2", "Z0"), ("Z0", "Z2", "Z1")]
    for b in range(2):
        i, mid, o = rot3[b]
        add(f"l3b{b}c1", ("l3_ws", 1 + 2 * b), 64, 64, 8, 1, 3, inb=i, outb=mid, evac="relu")
        add(f"l3b{b}c2", ("l3_ws", 2 + 2 * b), 64, 64, 8, 1, 3, inb=mid, outb=o,
            evac="res", idb=i, idkind="pad")
    return convs


S_ORDER3 = [0, 2, 3, 5, 6, 8, 1, 4, 7]  # kw==1 last


def d_ops(meta):
    """Yield (cb, s) in emission order. kw==1 shifts come last within each cb
    (they read the shifted shadow buffer, written after replication). For
    co=64, interleave the two halves so consecutive matmuls target
    alternating PE column groups."""
    ncb, k = meta["ncb"], meta["k"]
    s_order = S_ORDER3 if k == 3 else [0]
    if meta["co"] == 64:
        nh = ncb // 2
        for q in range(nh):
            for s in s_order:
                yield q, s
                yield nh + q, s
    else:
        for cb in range(ncb):
            for s in s_order:
                yield cb, s


def d_engine_map(meta):
    """op index (position in d_ops order) -> 'V'/'A'."""
    n_act = ENG_SPLIT.get(meta["name"], 0)
    ops = list(d_ops(meta))
    nsub = {16: BL // 2, 32: BL // 2, 64: 1}[meta["co"]]
    n_act_ops = n_act // nsub
    eng = {}
    if n_act_ops > 0:
        stride = max(1, len(ops) // n_act_ops)
        left = n_act_ops
        for i in range(len(ops)):
            if i % stride == 0 and left > 0:
                eng[i] = "A"
                left -= 1
            else:
                eng[i] = "V"
    else:
        eng = {i: "V" for i in range(len(ops))}
    return ops, eng, nsub


SCHED = make_schedule()
NWALL = sum(c["ncb"] * c["k"] * c["k"] for c in SCHED)
CST_OFF, CST_ONES, NCST = cst_layout()


# --------------------------------------------------------------------------
# host-side packing
# --------------------------------------------------------------------------
def get_w(inputs, wsrc):
    a = inputs[wsrc[0]]
    if len(wsrc) > 1:
        a = a[wsrc[1]]
    return a  # [co, ci, k, k]


def pack_host(inputs):
    wall = np.zeros((128, NWALL), np.float32)
    col = 0
    for m in SCHED:
        w = get_w(inputs, m["wsrc"])
        ci, g, k = m["ci"], m["g"], m["k"]
        for cb, s in d_ops(m):
            kh, kw = divmod(s, k)
            for gg in range(g):
                co = cb * g + gg
                wall[gg * ci:(gg + 1) * ci, col] = w[co, :, kh, kw]
            col += 1
    assert col == NWALL

    cst = np.zeros((128, NCST), np.float16)
    for (ci, g, co) in CST_VARIANTS:
        off = CST_OFF[(ci, g, co)]
        mblk = min(co, 32)
        ncb = co // g
        nper = mblk // g  # blocks per half-window
        for cb in range(ncb):
            q = cb % nper
            for gg in range(g):
                cst[gg * ci:(gg + 1) * ci, off + cb * mblk + q * g + gg] = -2.0
        oo = CST_ONES[(ci, g, co)]
        cst[:, oo:oo + co] = 1.0 / g

    stemw = inputs["conv1_w"].transpose(2, 3, 1, 0).reshape(27, 16).astype(np.float16)
    fcw = (inputs["fc_w"][:, :, 0, 0].T / 64.0).astype(np.float32)  # [64, 10]
    return wall, cst, stemw, fcw


# --------------------------------------------------------------------------
# graph builder
# --------------------------------------------------------------------------
_CACHE = {}


def build(debug=False):
    from concourse import bacc, mybir, tile

    F16, F32 = mybir.dt.float16, mybir.dt.float32
    A = mybir.AluOpType
    AF = mybir.ActivationFunctionType
    AX = mybir.AxisListType

    nc = bacc.Bacc("TRN2", target_bir_lowering=False, debug=False,
                   num_devices=CORES)
    xp_d = nc.dram_tensor("xp", [3, BL, 34, 34], F16, kind="ExternalInput")
    wall_d = nc.dram_tensor("wall", [128, NWALL], F32, kind="ExternalInput")
    cst_d = nc.dram_tensor("cst", [128, NCST], F16, kind="ExternalInput")
    stemw_d = nc.dram_tensor("stemw", [27, 16], F16, kind="ExternalInput")
    fcw_d = nc.dram_tensor("fcw", [64, 10], F32, kind="ExternalInput")
    out_d = nc.dram_tensor("out", [10, BL], F32, kind="ExternalOutput")
    dbg_d = {}
    if debug:
        for m in SCHED:
            shp = ([m["co"], BL, m["hout"] + 2, m["hout"] + 2]
                   if m["evac"] != "down" else [m["co"], BL, m["hout"], m["hout"]])
            dbg_d[m["name"]] = nc.dram_tensor(f'dbg_{m["name"]}', shp,
                                              F16, kind="ExternalOutput")
        dbg_d["stem"] = nc.dram_tensor("dbg_stem", [16, BL, 34, 34],
                                       F16, kind="ExternalOutput")

    with tile.TileContext(nc) as tc:
        import contextlib
        with contextlib.ExitStack() as ctx:
            pp = ctx.enter_context(tc.tile_pool(name="persist", bufs=1))
            dp = ctx.enter_context(tc.tile_pool(name="dtiles", bufs=6))
            sp = ctx.enter_context(tc.tile_pool(name="small", bufs=8))
            ppl = ctx.enter_context(tc.tile_pool(name="planes", bufs=6))
            psp = ctx.enter_context(tc.tile_pool(name="psum", bufs=8, space="PSUM"))
            drp = ctx.enter_context(tc.tile_pool(name="dram", bufs=4, space="DRAM"))

            wall = pp.tile([128, NWALL], F32, tag="wall")
            nwall = pp.tile([128, NWALL], F32, tag="nwall")
            cst = pp.tile([128, NCST], F16, tag="cst")
            stemw = pp.tile([27, 16], F16, tag="stemw")
            fcw = pp.tile([64, 10], F32, tag="fcw")
            epst = pp.tile([128, 1], F32, tag="epst")
            nc.sync.dma_start(wall[:], wall_d[:])
            nc.sync.dma_start(cst[:], cst_d[:])
            nc.sync.dma_start(stemw[:], stemw_d[:])
            nc.sync.dma_start(fcw[:], fcw_d[:])
            nc.vector.memset(epst[:], EPS)
            nc.vector.tensor_scalar(nwall[:], wall[:], -1.0, None, A.mult)

            # activation buffers (persistent, zeroed once => borders stay 0)
            bufs = {}
            for nm in ("X0", "X1", "X2"):
                bufs[nm] = pp.tile([128, BL, 34, 34], F16, name=nm, tag=nm)
            for nm in ("Y0", "Y1", "Y2"):
                bufs[nm] = pp.tile([128, BL, 18, 18], F16, name=nm, tag=nm)
            for nm in ("Z0", "Z1", "Z2"):
                bufs[nm] = pp.tile([128, BL, 10, 10], F16, name=nm, tag=nm)
            bufs["ID2"] = pp.tile([128, BL, 16, 16], F16, name="ID2", tag="ID2")
            bufs["ID3"] = pp.tile([128, BL, 8, 8], F16, name="ID3", tag="ID3")
            # shifted shadow copies (one column left) so kw==1 D-reads stay
            # 4-byte aligned for the DVE 4x mode
            for nm in ("X0", "X1", "X2"):
                bufs[nm + "s"] = pp.tile([128, BL, 34, 33], F16, name=nm + "s",
                                         tag=nm + "s")
            for nm in ("Y0", "Y1", "Y2"):
                bufs[nm + "s"] = pp.tile([128, BL, 18, 17], F16, name=nm + "s",
                                         tag=nm + "s")
            for nm in ("Z0", "Z1", "Z2"):
                bufs[nm + "s"] = pp.tile([128, BL, 10, 9], F16, name=nm + "s",
                                         tag=nm + "s")
            for nm in ("X0", "X1", "X2", "Y0", "Y1", "Y2", "Z0", "Z2", "Z1"):
                nc.vector.memset(bufs[nm][:], 0.0)
                nc.vector.memset(bufs[nm + "s"][:], 0.0)

            # round-robin DMA queue picker for replication copies
            rq_engines = None
            rq_i = [0]

            def rqueue():
                e = rq_engines[rq_i[0] % len(rq_engines)]
                rq_i[0] += 1
                return e
            rq_engines = [nc.sync, nc.gpsimd]

            # ---------------- BN helpers ----------------
            def bn_finish(gred, n, rr, rows):
                """gred: [rows, 2] (S1, S2) global sums tile. rr: [rows, 2]
                out (r, -m*r). ACT-heavy to minimize engine switches."""
                mt = sp.tile([rows, 4], F32, tag="bnm", name="bnm")
                nc.scalar.activation(rr[:, 2:3], gred[:, 0:1], AF.Identity,
                                     scale=1.0 / n)                  # m
                nc.scalar.activation(mt[:, 1:2], rr[:, 2:3], AF.Square)  # m^2
                nc.vector.tensor_scalar(mt[:, 2:3], gred[:, 1:2], 1.0 / n,
                                        mt[:, 1:2], A.mult, A.subtract)
                nc.scalar.activation(mt[:, 3:4], mt[:, 2:3], AF.Sqrt,
                                     bias=epst[0:rows, 0:1])
                nc.vector.reciprocal(rr[:, 0:1], mt[:, 3:4])
                nc.vector.tensor_scalar(rr[:, 1:2], rr[:, 2:3], -1.0,
                                        rr[:, 0:1], A.mult, A.mult)

            def allgather(st_tiles):
                """st_tiles: list of (tile, nelem_f32). Returns DRAM agout tile
                + per-input offset list. agout layout: [8 ranks, sum(nelem)]."""
                tot = sum(n for _, n in st_tiles)
                sin = drp.tile([tot], F32, tag="agi", name="agi")
                offs = []
                o = 0
                for t, n in st_tiles:
                    nc.sync.dma_start(sin[o:o + n], t[:])
                    offs.append(o)
                    o += n
                sout = drp.tile([CORES, tot], F32, tag="ago", name="ago")
                nc.gpsimd.collective_compute(
                    "AllGather", A.bypass,
                    replica_groups=[list(range(CORES))],
                    ins=[sin.opt()], outs=[sout.opt()],
                )
                return sout, offs

            # ---------------- per-layout helpers ----------------
            # layouts keyed by co: how psum / stats / evac are organized.
            def psum_alloc(meta, name):
                co = meta["co"]
                if co == 16:
                    return [psp.tile([128, 512], F32, tag="ps", name=f"{name}_b{b}")
                            for b in range(2)]
                if co == 32:
                    return [psp.tile([128, 256], F32, tag="ps", name=f"{name}_b0")]
                return [psp.tile([64, 256], F32, tag="ps", name=f"{name}_b0")]

            def emit_bank_stats16(st, psum, b):
                jk = dp.tile([128, 512], F16, tag="junk", name="junk")
                nc.vector.tensor_scalar(jk[:], psum[:], 0.0, None,
                                        A.add, A.add,
                                        accum_out=st[:, 2 * b:2 * b + 1])
                nc.scalar.activation(jk[:], psum[:], AF.Square,
                                     accum_out=st[:, 2 * b + 1:2 * b + 2])

            def stats_emit(meta, psums, st):
                co = meta["co"]
                if co == 16:
                    for b in range(2):
                        emit_bank_stats16(st, psums[b], b)
                elif co == 32:
                    jk = dp.tile([128, 256], F16, tag="junk", name="junk")
                    nc.vector.tensor_scalar(jk[:], psums[0][:], 0.0, None,
                                            A.add, A.add, accum_out=st[:, 0:1])
                    nc.scalar.activation(jk[:], psums[0][:], AF.Square,
                                         accum_out=st[:, 1:2])
                else:
                    jk = dp.tile([64, 256], F16, tag="junk64", name="junk")
                    nc.vector.tensor_scalar(jk[:], psums[0][:], 0.0, None,
                                            A.add, A.add, accum_out=st[:, 0:1])
                    nc.scalar.activation(jk[:], psums[0][:], AF.Square,
                                         accum_out=st[:, 1:2])

            def st_alloc(meta):
                co = meta["co"]
                if co == 16:
                    return sp.tile([128, 4], F32, tag="st4", name=f"st_{meta['name']}"), 384
                if co == 32:
                    return sp.tile([128, 3], F32, tag="st", name=f"st_{meta['name']}"), 384
                return sp.tile([64, 3], F32, tag="st64", name=f"st_{meta['name']}"), 192

            def st_finalize(meta, st):
                """For co16: combine the two banks' partial stats -> [128, 3]."""
                if meta["co"] != 16:
                    return st
                st2 = sp.tile([128, 3], F32, tag="st", name="st2")
                nc.vector.tensor_tensor(st2[:, 0:2], st[:, 0:2], st[:, 2:4], A.add)
                return st2

            def gather_reduce(meta, sout, off):
                """Gather the AG output into per-channel layout + reduce + bn.
                Blob layout per rank: flat st2 [rows, 2] (row-major).
                Returns rr tile ([128,2] for co<=32 replicated, [64,2] co=64)."""
                co = meta["co"]
                n = GB * meta["hout"] * meta["hout"]
                if co == 64:
                    gst = sp.tile([64, 8, 2], F32, tag="gst64", name="gst")
                    sv = sout[:, off:off + 192].rearrange(
                        "r (i k) -> i r k", i=64, k=3)[:, :, 0:2]
                    nc.sync.dma_start(gst[:], sv)
                    red = sp.tile([64, 2], F32, tag="red64", name="red")
                    nc.vector.tensor_reduce(
                        red[:], gst[:, :, :].rearrange("p r k -> p k r"),
                        AX.X, A.add)
                    rr = sp.tile([64, 3], F32, tag="rr64", name="rr")
                    bn_finish(red, n, rr, 64)
                    return rr
                nch = co  # channels live at rows 32j+0:co
                gst = sp.tile([nch, 4, 8, 2], F32, tag="gst", name="gst")
                for j in range(4):
                    sv = sout[:, off + 96 * j:off + 96 * j + 3 * nch].rearrange(
                        "r (i k) -> i r k", i=nch, k=3)[:, :, 0:2]
                    (nc.sync if j % 2 == 0 else nc.gpsimd).dma_start(
                        gst[:, j, :, :], sv)
                red = sp.tile([nch, 2], F32, tag="red", name="red")
                nc.vector.tensor_reduce(
                    red[:], gst[:, :, :, :].rearrange("p j r k -> p k (j r)"),
                    AX.X, A.add)
                rr = sp.tile([128, 3], F32, tag="rr", name="rr")
                bn_finish(red, n, rr[0:nch, :], nch)
                for t, eng in ((1, nc.scalar), (2, nc.gpsimd), (3, nc.sync)):
                    eng.dma_start(rr[32 * t:32 * t + nch, :], rr[0:nch, :])
                return rr

            def evacuate(meta, psums, rr):
                """psum -> xout (+ per-image replication)."""
                co, hout = meta["co"], meta["hout"]
                xout = bufs[meta["outb"]]
                kind = meta["evac"]
                idt = bufs[meta["idb"]] if kind == "res" else None
                if co == 16:
                    for j in range(BL):
                        for b in range(2):
                            rg = 32 * (2 * (j % 2) + b)
                            ps = psums[j // 2][rg:rg + 16, :]
                            ov = xout[rg:rg + 16, j,
                                      1 + 16 * b:17 + 16 * b, 1:33]
                            rrs = rr[rg:rg + 16, :]
                            if kind == "res":
                                idv = idt[rg:rg + 16, j,
                                          1 + 16 * b:17 + 16 * b, 1:33]
                                t = dp.tile([128, 512], F16, tag="tres", name="tres")
                                ts = t[rg:rg + 16, :]
                                nc.vector.scalar_tensor_tensor(
                                    ts, ps, rrs[:, 0:1], idv, A.mult, A.add)
                                if (j + b) % 2 == 0:
                                    nc.scalar.activation(ov, ts, AF.Relu,
                                                         bias=rrs[:, 1:2])
                                else:
                                    nc.vector.tensor_scalar(
                                        ov, ts, rrs[:, 1:2], 0.0, A.add, A.max)
                            else:
                                if (j + b) % 2 == 0:
                                    nc.scalar.activation(ov, ps, AF.Relu,
                                                         bias=rrs[:, 1:2],
                                                         scale=rrs[:, 0:1])
                                else:
                                    # relu(bn(u)) = (max(u,m)-m)*r on DVE
                                    t1 = dp.tile([128, 512], F16, tag="tres",
                                                 name="t1")
                                    nc.vector.tensor_scalar(
                                        t1[rg:rg + 16, :], ps, rrs[:, 2:3],
                                        rrs[:, 2:3], A.max, A.subtract)
                                    nc.vector.tensor_scalar(
                                        ov, t1[rg:rg + 16, :], rrs[:, 0:1],
                                        None, A.mult)
                        # assemble + replicate image j to all 8 groups:
                        # halves evacuated to row-groups rb (top) and rb+32
                        # (bottom) -> cross-copy, double, then 64->64
                        rb = 64 * (j % 2)
                        rqueue().dma_start(xout[rb:rb + 16, j, 17:33, :],
                                           xout[rb + 32:rb + 48, j, 17:33, :])
                        rqueue().dma_start(xout[rb + 32:rb + 48, j, 0:17, :],
                                           xout[rb:rb + 16, j, 0:17, :])
                        rqueue().dma_start(xout[rb + 16:rb + 32, j, :, :],
                                           xout[rb:rb + 16, j, :, :])
                        rqueue().dma_start(xout[rb + 48:rb + 64, j, :, :],
                                           xout[rb + 32:rb + 48, j, :, :])
                        ro = (rb + 64) % 128
                        rqueue().dma_start(xout[ro:ro + 64, j, :, :],
                                           xout[rb:rb + 64, j, :, :])
                        xsh = bufs.get(meta["outb"] + "s")
                        if xsh is not None:
                            rqueue().dma_start(xsh[:, j, :, 0:33],
                                               xout[:, j, :, 1:34])
                elif co == 32:
                    hp = hout + 2
                    for c in range(BL):
                        ps = psums[0][32 * c:32 * c + 32, :]
                        rrs = rr[32 * c:32 * c + 32, :]
                        if kind == "down":
                            ov = bufs["ID2"][32 * c:32 * c + 32, c, :, :]
                            if c % 2 == 0:
                                nc.scalar.activation(ov, ps, AF.Identity,
                                                     bias=rrs[:, 1:2],
                                                     scale=rrs[:, 0:1])
                            else:
                                nc.vector.tensor_scalar(
                                    ov, ps, rrs[:, 0:1], rrs[:, 1:2],
                                    A.mult, A.add)
                            src = bufs["ID2"][32 * c:32 * c + 32, c, :, :]
                            dstbuf = bufs["ID2"]
                            sh = [hout, hout]
                        else:
                            ov = xout[32 * c:32 * c + 32, c, 1:1 + hout, 1:1 + hout]
                            if kind == "res":
                                if meta["idkind"] == "pad":
                                    idv = idt[32 * c:32 * c + 32, c,
                                              1:1 + hout, 1:1 + hout]
                                else:
                                    idv = idt[32 * c:32 * c + 32, c, :, :]
                                t = dp.tile([128, 256], F16, tag="tres32", name="tres")
                                ts = t[32 * c:32 * c + 32, :]
                                nc.vector.scalar_tensor_tensor(
                                    ts, ps, rrs[:, 0:1], idv, A.mult, A.add)
                                if c % 2 == 0:
                                    nc.scalar.activation(ov, ts, AF.Relu,
                                                         bias=rrs[:, 1:2])
                                else:
                                    nc.vector.tensor_scalar(
                                        ov, ts, rrs[:, 1:2], 0.0, A.add, A.max)
                            else:
                                if c % 2 == 0:
                                    nc.scalar.activation(ov, ps, AF.Relu,
                                                         bias=rrs[:, 1:2],
                                                         scale=rrs[:, 0:1])
                                else:
                                    t1 = dp.tile([128, 256], F16, tag="tres32",
                                                 name="t1")
                                    nc.vector.tensor_scalar(
                                        t1[32 * c:32 * c + 32, :], ps,
                                        rrs[:, 2:3], rrs[:, 2:3],
                                        A.max, A.subtract)
                                    nc.vector.tensor_scalar(
                                        ov, t1[32 * c:32 * c + 32, :],
                                        rrs[:, 0:1], None, A.mult)
                            src = xout[32 * c:32 * c + 32, c, :, :]
                            dstbuf = xout
                            sh = [hp, hp]
                        rb = 32 * c
                        rp = rb ^ 32
                        rqueue().dma_start(dstbuf[rp:rp + 32, c, :, :], src)
                        rh = rb // 64 * 64
                        ro = rh ^ 64
                        rqueue().dma_start(dstbuf[ro:ro + 64, c, :, :],
                                           dstbuf[rh:rh + 64, c, :, :])
                        if kind != "down":
                            xsh = bufs.get(meta["outb"] + "s")
                            if xsh is not None:
                                rqueue().dma_start(xsh[:, c, :, 0:17],
                                                   xout[:, c, :, 1:18])
                else:  # co == 64
                    ps = psums[0][:, :]
                    if kind == "down":
                        ov = bufs["ID3"][0:64, :, :, :]
                        nc.scalar.activation(ov, ps, AF.Identity,
                                             bias=rr[:, 1:2], scale=rr[:, 0:1])
                        nc.sync.dma_start(bufs["ID3"][64:128, :, :, :],
                                          bufs["ID3"][0:64, :, :, :])
                    else:
                        nim = hout * hout
                        if kind == "res":
                            t = dp.tile([64, 256], F16, tag="tres64", name="tres")
                            for b in range(BL):
                                if meta["idkind"] == "pad":
                                    idv = idt[0:64, b, 1:1 + hout, 1:1 + hout]
                                else:
                                    idv = idt[0:64, b, :, :]
                                nc.vector.scalar_tensor_tensor(
                                    t[:, nim * b:nim * b + nim],
                                    psums[0][:, nim * b:nim * b + nim],
                                    rr[:, 0:1], idv, A.mult, A.add)
                            for b in range(BL):
                                ovb = xout[0:64, b, 1:1 + hout, 1:1 + hout]
                                if b % 2 == 0:
                                    nc.scalar.activation(
                                        ovb, t[:, nim * b:nim * b + nim],
                                        AF.Relu, bias=rr[:, 1:2])
                                else:
                                    nc.vector.tensor_scalar(
                                        ovb, t[:, nim * b:nim * b + nim],
                                        rr[:, 1:2], 0.0, A.add, A.max)
                        else:
                            t1 = dp.tile([64, 256], F16, tag="tres64", name="t1")
                            for b in range(BL):
                                ovb = xout[0:64, b, 1:1 + hout, 1:1 + hout]
                                if b % 2 == 0:
                                    nc.scalar.activation(
                                        ovb, psums[0][:, nim * b:nim * b + nim],
                                        AF.Relu, bias=rr[:, 1:2],
                                        scale=rr[:, 0:1])
                                else:
                                    nc.vector.tensor_scalar(
                                        t1[:, nim * b:nim * b + nim],
                                        psums[0][:, nim * b:nim * b + nim],
                                        rr[:, 2:3], rr[:, 2:3],
                                        A.max, A.subtract)
                                    nc.vector.tensor_scalar(
                                        ovb, t1[:, nim * b:nim * b + nim],
                                        rr[:, 0:1], None, A.mult)
                        nc.sync.dma_start(xout[64:128, :, :, :],
                                          xout[0:64, :, :, :])
                        xsh = bufs.get(meta["outb"] + "s")
                        if xsh is not None:
                            rqueue().dma_start(xsh[:, :, :, 0:9],
                                               xout[:, :, :, 1:10])
                if debug and meta["name"] in dbg_d:
                    if kind == "down":
                        db = bufs["ID2"] if co == 32 else bufs["ID3"]
                        nc.sync.dma_start(dbg_d[meta["name"]][:], db[0:co])
                    else:
                        nc.sync.dma_start(dbg_d[meta["name"]][:], xout[0:co])

            # ---------------- parity planes for stride-2 convs ----------------
            def make_planes(meta):
                """Precompact stride-2 input into 4 stride-1 parity planes."""
                xin = bufs[meta["inb"]]
                hin = meta["hin"]          # 32 or 16
                hh = hin // 2 + 1          # 17 or 9
                wpl = hh + 1               # even width
                planes = {}
                engs = [nc.vector, nc.gpsimd, nc.vector, nc.gpsimd]
                i = 0
                for pr in (0, 1):
                    for pc in (0, 1):
                        pl = ppl.tile([128, BL, hh, wpl], F16,
                                      tag=f"pl{hin}", name=f"pl{pr}{pc}")
                        src = xin[:, :, pr:pr + 2 * hh - 1:2, pc:pc + 2 * hh - 1:2]
                        engs[i % 4].tensor_scalar(pl[:, :, :, 0:hh], src, 0.0,
                                                  None, A.add)
                        i += 1
                        planes[(pr, pc)] = pl
                return planes

            # ---------------- adder conv core ----------------
            wall_col = [0]

            def adder_conv(meta, planes=None, stats_st=None):
                ci, co, g, ncb, k = meta["ci"], meta["co"], meta["g"], meta["ncb"], meta["k"]
                hout, stride = meta["hout"], meta["stride"]
                mblk = meta["mblk"]
                xin = bufs[meta["inb"]]
                coff = CST_OFF[CST_VARIANTS[meta["cst"]]]
                ones_off = CST_ONES[CST_VARIANTS[meta["cst"]]]
                psums = psum_alloc(meta, meta["name"])
                ops, engmap, nsub = d_engine_map(meta)
                ncol = {16: 512, 32: 256, 64: 256}[co]
                nh = ncb // 2 if co == 64 else None

                def xview(kh, kw, sub):
                    if stride == 2:
                        if k == 1:
                            pl, r0, c0 = planes[(1, 1)], 0, 0
                        else:
                            pl = planes[(kh % 2, kw % 2)]
                            r0, c0 = kh // 2, kw // 2
                        if co == 32 and sub is not None:  # per image-pair
                            p, = sub
                            return pl[:, 2 * p:2 * p + 2, r0:r0 + hout, c0:c0 + hout]
                        return pl[:, :, r0:r0 + hout, c0:c0 + hout]
                    if co == 16:
                        j, = sub
                        return xin[:, j, kh:kh + hout, kw:kw + hout]
                    if co == 32:
                        if sub is None:
                            return xin[:, :, kh:kh + hout, kw:kw + hout]
                        p, = sub
                        return xin[:, 2 * p:2 * p + 2, kh:kh + hout, kw:kw + hout]
                    return xin[:, :, kh:kh + hout, kw:kw + hout]

                def emit_d(eng, dv, xv, col):
                    # max-form: D = max(x, w) (DVE/GPS) or relu(x - w) (ACT);
                    # blockdiag(-2) + a sum-x ones matmul recovers -sum|x-w|
                    # up to a per-channel constant absorbed by BN.
                    if eng == "A":
                        nc.scalar.activation(dv, xv, AF.Relu,
                                             bias=nwall[:, col:col + 1])
                    elif eng == "G":
                        nc.gpsimd.tensor_scalar(dv, xv, wall[:, col:col + 1],
                                                None, A.max)
                    else:
                        nc.vector.tensor_scalar(dv, xv, wall[:, col:col + 1],
                                                None, A.max)

                dshape = {16: [128, hout, hout], 32: [128, 2, hout, hout],
                          64: [128, BL, hout, hout]}[co]
                dtag = f"d{co}_{hout}"

                if co == 16:
                    # pair-split: all ops for images {0,1} (bank 0), then
                    # images {2,3} (bank 1); per-bank stats emitted inline so
                    # the AllGather can start while pair 1 computes.
                    colbase = wall_col[0]
                    wall_col[0] += len(ops)
                    for p in range(2):
                        for oi, (cb, s) in enumerate(ops):
                            kh, kw = divmod(s, k)
                            col = colbase + oi
                            eng = engmap[oi]
                            lhsT = cst[:, coff + cb * mblk:coff + (cb + 1) * mblk]
                            first = oi == 0
                            last = oi == len(ops) - 1
                            d = dp.tile(dshape, F16, tag=dtag, name="d")
                            emit_d(eng, d[:], xview(kh, kw, (p,)), col)
                            for ii in range(2):
                                for b in range(2):
                                    rg = 32 * (2 * ii + b)
                                    nc.tensor.matmul(
                                        psums[p][rg:rg + 16, :], lhsT,
                                        d[:, ii, 16 * b:16 * b + 16, :],
                                        start=first, stop=last,
                                        tile_position=(0, rg))
                            if cb == 0:
                                xv = xview(kh, kw, (p,))
                                for ii in range(2):
                                    for b in range(2):
                                        rg = 32 * (2 * ii + b)
                                        nc.tensor.matmul(
                                            psums[p][rg:rg + 16, :],
                                            cst[:, ones_off:ones_off + 16],
                                            xv[:, ii, 16 * b:16 * b + 16, :],
                                            start=False, stop=False,
                                            tile_position=(0, rg))
                        if stats_st is not None:
                            emit_bank_stats16(stats_st, psums[p], p)
                    return psums

                for oi, (cb, s) in enumerate(ops):
                    kh, kw = divmod(s, k)
                    col = wall_col[0]
                    wall_col[0] += 1
                    eng = engmap[oi]
                    lhsT = cst[:, coff + cb * mblk:coff + (cb + 1) * mblk]
                    if co == 64:
                        h = cb // nh
                        first = (cb % nh == 0) and s == 0
                        last = (cb % nh == nh - 1) and s == k * k - 1
                        d = dp.tile(dshape, F16, tag=dtag, name="d")
                        emit_d(eng, d[:], xview(kh, kw, ()), col)
                        nc.tensor.matmul(
                            psums[0][32 * h:32 * h + 32, :], lhsT, d[:],
                            start=first, stop=last, tile_position=(0, 32 * h))
                        if cb == nh:  # both halves started: sum-x correction
                            nc.tensor.matmul(
                                psums[0][0:64, :],
                                cst[:, ones_off:ones_off + 64],
                                xview(kh, kw, ()),
                                start=False, stop=False, tile_position=(0, 0))
                    elif co == 32:
                        first = cb == 0 and s == 0
                        last = cb == ncb - 1 and s == k * k - 1
                        for p in range(2):
                            d = dp.tile(dshape, F16, tag=dtag, name="d")
                            emit_d(eng, d[:], xview(kh, kw, (p,)), col)
                            for ii in range(2):
                                c = 2 * p + ii
                                nc.tensor.matmul(
                                    psums[0][32 * c:32 * c + 32, :], lhsT,
                                    d[:, ii, :, :],
                                    start=first, stop=last,
                                    tile_position=(0, 32 * c))
                        if cb == 0:
                            xv = xview(kh, kw, None)
                            for c in range(BL):
                                nc.tensor.matmul(
                                    psums[0][32 * c:32 * c + 32, :],
                                    cst[:, ones_off:ones_off + 32],
                                    xv[:, c, :, :],
                                    start=False, stop=False,
                                    tile_position=(0, 32 * c))
                    else:  # co == 16
                        pass  # handled in the pair loop below
                return psums

            def conv_tail(meta, psums, st, nst):
                if meta["co"] != 16:
                    stats_emit(meta, psums, st)
                st = st_finalize(meta, st)
                sout, offs = allgather([(st, nst)])
                rr = gather_reduce(meta, sout, offs[0])
                evacuate(meta, psums, rr)

            # ---------------- stem ----------------
            with nc.named_scope("stem"):
                pt = pp.tile([27, BL, 32, 32], F16, tag="pt")
                for s in range(9):
                    kh, kw = divmod(s, 3)
                    nc.sync.dma_start(pt[3 * s:3 * s + 3],
                                      xp_d[:, :, kh:kh + 32, kw:kw + 32])
                m_stem = conv_meta(16, 16, 32, 1, 3)
                m_stem.update(outb="X0", evac="relu", name="stem")
                ps_stem = psum_alloc(m_stem, "stem")
                st, nst = st_alloc(m_stem)
                for p in range(2):
                    for ii in range(2):
                        j = 2 * p + ii
                        for b in range(2):
                            rg = 32 * (2 * ii + b)
                            nc.tensor.matmul(
                                ps_stem[p][rg:rg + 16, :], stemw[:],
                                pt[:, j, 16 * b:16 * b + 16, :],
                                start=True, stop=True, tile_position=(0, rg))
                    emit_bank_stats16(st, ps_stem[p], p)
                st = st_finalize(m_stem, st)
                sout, offs = allgather([(st, nst)])
                rr = gather_reduce(m_stem, sout, offs[0])
                evacuate(m_stem, ps_stem, rr)
                if debug:
                    nc.sync.dma_start(dbg_d["stem"][:], bufs["X0"][0:16])

            # ---------------- adder conv layers ----------------
            i = 0
            while i < len(SCHED):
                meta = SCHED[i]
                if meta.get("grp"):  # merged transition pair (tc1 + td)
                    meta2 = SCHED[i + 1]
                    with nc.named_scope(meta["name"]):
                        planes = make_planes(meta)
                        ps1 = adder_conv(meta, planes)
                    with nc.named_scope(meta2["name"]):
                        ps2 = adder_conv(meta2, planes)
                        st1, n1 = st_alloc(meta)
                        st2, n2 = st_alloc(meta2)
                        stats_emit(meta, ps1, st1)
                        stats_emit(meta2, ps2, st2)
                        st1 = st_finalize(meta, st1)
                        st2 = st_finalize(meta2, st2)
                        sout, offs = allgather([(st1, n1), (st2, n2)])
                        rr1 = gather_reduce(meta, sout, offs[0])
                        rr2 = gather_reduce(meta2, sout, offs[1])
                        evacuate(meta, ps1, rr1)
                        evacuate(meta2, ps2, rr2)
                    i += 2
                else:
                    with nc.named_scope(meta["name"]):
                        st, nst = st_alloc(meta)
                        ps = adder_conv(meta, stats_st=st)
                        conv_tail(meta, ps, st, nst)
                    i += 1

            # ---------------- avgpool + fc + final bn ----------------
            with nc.named_scope("fc"):
                zf = bufs[SCHED[-1]["outb"]]
                pooled = sp.tile([64, BL], F32, tag="pool", name="pooled")
                junkp = dp.tile([64, 64], F16, tag="junkp", name="junkp")
                for b in range(BL):
                    nc.scalar.activation(junkp[:], zf[0:64, b, 1:9, 1:9],
                                         AF.Identity,
                                         accum_out=pooled[:, b:b + 1])
                ps_fc = psp.tile([10, BL], F32, tag="ps", name="ps_fc")
                nc.tensor.matmul(ps_fc[:, :], fcw[:], pooled[:], start=True, stop=True)
                st = sp.tile([10, 2], F32, tag="stfc", name="st_fc")
                junk = dp.tile([10, BL], F16, tag="junkfc", name="junk_fc")
                nc.scalar.activation(junk[:], ps_fc[:], AF.Identity,
                                     accum_out=st[:, 0:1])
                nc.scalar.activation(junk[:], ps_fc[:], AF.Square,
                                     accum_out=st[:, 1:2])
                sout, offs = allgather([(st, 20)])
                gst = sp.tile([10, 8, 2], F32, tag="gstfc", name="gst_fc")
                sv = sout[:, 0:20].rearrange("r (i k) -> i r k", i=10, k=2)
                nc.sync.dma_start(gst[:], sv)
                red = sp.tile([10, 2], F32, tag="redfc", name="red_fc")
                nc.vector.tensor_reduce(
                    red[:], gst[:, :, :].rearrange("p r k -> p k r"),
                    mybir.AxisListType.X, A.add)
                rr = sp.tile([10, 3], F32, tag="rrfc", name="rr_fc")
                bn_finish(red, GB, rr, 10)
                osb = sp.tile([10, BL], F32, tag="osb", name="osb")
                nc.scalar.activation(osb[:], ps_fc[:], AF.Identity,
                                     bias=rr[:, 1:2], scale=rr[:, 0:1])
                nc.sync.dma_start(out_d[:], osb[:])

    nc.compile()
    return nc


def get_nc(debug=False):
    key = f"nc{debug}"
    if key not in _CACHE:
        _CACHE[key] = build(debug)
    return _CACHE[key]


# --------------------------------------------------------------------------
# entry point
# --------------------------------------------------------------------------
def kernel(**inputs):
    from concourse.bass_utils import run_bass_kernel_spmd

    x = inputs["x"]  # [32, 3, 32, 32] f32
    wall, cst, stemw, fcw = pack_host(inputs)
    xpad = np.zeros((CORES, 3, BL, 34, 34), np.float16)
    xs = x.reshape(CORES, BL, 3, 32, 32).transpose(0, 2, 1, 3, 4)
    xpad[:, :, :, 1:33, 1:33] = xs.astype(np.float16)

    nc = get_nc()
    in_maps = [{"xp": xpad[i], "wall": wall, "cst": cst,
                "stemw": stemw, "fcw": fcw} for i in range(CORES)]
    res = run_bass_kernel_spmd(nc, in_maps, list(range(CORES)))
    out = np.concatenate([r["out"].T for r in res.results], axis=0)
    return out.astype(np.float32)
